# revision 36
# baseline (speedup 1.0000x reference)
"""Bass/Trainium2 kernel for nn_MOEFeedForward (8-expert top-2 MoE + shared expert).

Sharding: expert-parallel with host-side dispatch. The host computes the gate
(softmax + top-2) and routes tokens: core c receives expert c's tokens (padded
to capacity A = ceil8(max expert load)) plus a 1/8 token-slice of the
shared-expert work (B = 256 tokens). Every core runs A+B token-FFN columns of
identical shape (hid=2048, dim=768) — balanced, no 8x dense overcompute. The
host applies the gate weights and scatter-adds per-core outputs into the full
result.

Device kernel: all operands pre-transposed/laid out on the host so the device
does only contiguous DMAs and back-to-back bf16 matmuls at 1 col/cycle.
Column pieces of <=512 run mm1/mm3 (hid-chunked, PSUM-accumulated over the 6
dim-chunks), silu*mul drains to bf16 hT, then mm2 in d-major form
(y[d, t], 6 dim psums contracting 16 hid chunks). Dummy PE warmup matmuls
ramp the tensor-engine clock while the first DMAs land. Cost-model makespan
~105.8us/core vs ~99.8us pure-matmul floor at 2.4 GHz.

Self-contained: hardcodes shapes from the problem spec.
"""
import os
import sys

sys.path.insert(0, "/opt/trn_rl_repo")

from contextlib import ExitStack

import numpy as np
from ml_dtypes import bfloat16

import concourse.bass as bass
import concourse.tile as tile
from concourse import mybir
from concourse.bass_utils import run_bass_kernel_spmd
from concourse.vector_clock import ScopedClock

DIM = 768
HID = 2048
E = 8
T = 2048
N_CORES = 8
B_SH = T // N_CORES  # shared-expert tokens per core (256)
DC = DIM // 128      # 6 d-chunks
HC = HID // 128      # 16 hid-chunks

F32 = mybir.dt.float32
BF16 = mybir.dt.bfloat16

AF = mybir.ActivationFunctionType
OP = mybir.AluOpType


# ---------------------------------------------------------------------------
# Walrus in this container rejects CTRL instructions (NoOp/Drain) carrying
# more than one sem wait. TileContext's tail drain carries one wait per
# outstanding semaphore. Replace it with a chain of SP nops (one wait each)
# followed by a bare drain.
def _patched_drain_and_barrier(self, tick_clock, wait_clock):
    import bass_rust

    nop_inst = self.nc.sync.nop(nofuse=True, hint="pre_drain_wait_funnel")
    wait_clock.add_sem_waits(
        nop_inst.ins, ScopedClock({None: tick_clock.global_clock})
    )
    si = nop_inst.ins.sync_info
    waits = list(si.on_wait) if si else []
    if len(waits) > 1:
        nop_inst.ins.sync_info.on_wait = waits[:1]
        for w in waits[1:]:
            extra = self.nc.sync.nop(nofuse=True, hint="pre_drain_wait_funnel")
            extra.ins.sync_info = bass_rust.SyncInfo(on_wait=[w], on_update=[])
    self.nc.sync.drain()

    self.nc.all_engine_barrier()
    assert self.sems is not None
    popped = self.nc._tile_sem_poison_stack.pop()
    assert popped is self._sem_poison
    self.nc.clear_and_free_semaphores(list(self.sems.allocated().values()))
    self.nc.all_engine_barrier()


tile.TileContext._drain_and_barrier = _patched_drain_and_barrier


def _split_multi_waits(nc, max_waits=1):
    """This walrus build allows at most one sem wait per instruction. Hoist
    extra waits onto same-engine nops inserted immediately before."""
    import bass_rust

    n_split = 0
    for f in nc.m.functions:
        for bb in f.blocks:
            il = bb.instructions
            i = 0
            while i < len(il):
                inst = il[i]
                si = inst.sync_info
                if si is None or len(si.on_wait) <= max_waits:
                    i += 1
                    continue
                waits = list(si.on_wait)
                si.on_wait = waits[:max_waits]
                for k, w in enumerate(waits[max_waits:]):
                    nop = mybir.InstNoOp(
                        name=f"{inst.name}-wsplit{k}", ins=[], outs=[]
                    )
                    nop.engine = inst.engine
                    nop.sync_info = bass_rust.SyncInfo(on_wait=[w], on_update=[])
                    il.insert(i, nop)
                    i += 1
                n_split += 1
                i += 1
    return n_split
# ---------------------------------------------------------------------------


def _build_kernel(A, reps=1, F=0, B=B_SH):
    """A: expert-token capacity. Columns [0, A) use the expert weight set;
    with F>0, columns [A, A+F) use a per-core 'flex' weight set (host fills
    with either this core's expert weights or the shared weights); columns
    [A+F, A+F+B) use the shared weight set.
    Output y is d-major: y_d[p, dc, t] = y[t, dc*128+p].
    reps>1 repeats the whole compute (for benchmarking)."""
    N = A + F + B
    nc = bass.Bass()
    xT_d = nc.dram_tensor("xT", [128, DC, N], BF16, kind="ExternalInput")
    w1_d = nc.dram_tensor("w1T", [128, HC, DC, 128], BF16, kind="ExternalInput")
    w3_d = nc.dram_tensor("w3T", [128, HC, DC, 128], BF16, kind="ExternalInput")
    w2_d = nc.dram_tensor("w2T", [128, HC, DC, 128], BF16, kind="ExternalInput")
    s1_d = nc.dram_tensor("s1T", [128, HC, DC, 128], BF16, kind="ExternalInput")
    s3_d = nc.dram_tensor("s3T", [128, HC, DC, 128], BF16, kind="ExternalInput")
    s2_d = nc.dram_tensor("s2T", [128, HC, DC, 128], BF16, kind="ExternalInput")
    if F:
        f1_d = nc.dram_tensor("f1T", [128, HC, DC, 128], BF16, kind="ExternalInput")
        f3_d = nc.dram_tensor("f3T", [128, HC, DC, 128], BF16, kind="ExternalInput")
        f2_d = nc.dram_tensor("f2T", [128, HC, DC, 128], BF16, kind="ExternalInput")
    y_d = nc.dram_tensor("y", [128, DC, N], F32, kind="ExternalOutput")

    # column pieces: (start, len, weight-set)
    first = int(os.environ.get("K_P0", "512"))
    exp_pieces = []
    c0 = 0
    while c0 < A:
        ln = min(first if c0 == 0 else 512, A - c0)
        exp_pieces.append((c0, ln, 0))
        c0 += ln
    # flex is processed LAST: its weights reuse w1/w3/w2's SBUF (same pool
    # tags), so their DMA can only land after the expert pieces finish.
    pieces = exp_pieces + [(A + F, B, 1)]
    if F:
        pieces = pieces + [(A, F, 2)]

    _g = lambda k, d: int(os.environ.get(k, str(d)))
    with tile.TileContext(nc) as tc, ExitStack() as ctx:
        persist = ctx.enter_context(tc.tile_pool(name="persist", bufs=1))
        silu_p = ctx.enter_context(tc.tile_pool(name="silu", bufs=_g("K_SILU", 3)))
        yo_p = ctx.enter_context(tc.tile_pool(name="yo", bufs=_g("K_YO", 3)))
        h_ps = ctx.enter_context(
            tc.tile_pool(name="h_ps", bufs=_g("K_HPS", 4), space="PSUM"))
        y_ps = ctx.enter_context(
            tc.tile_pool(name="y_ps", bufs=_g("K_YPS", 3), space="PSUM"))
        wup_ps = ctx.enter_context(tc.tile_pool(name="wup_ps", bufs=1, space="PSUM"))

        xT = persist.tile([128, DC, N], BF16, tag="xT")
        w1T = persist.tile([128, HC, DC, 128], BF16, tag="w1T")
        w3T = persist.tile([128, HC, DC, 128], BF16, tag="w3T")
        s1T = persist.tile([128, HC, DC, 128], BF16, tag="s1T")
        s3T = persist.tile([128, HC, DC, 128], BF16, tag="s3T")
        w2T = persist.tile([128, HC, DC, 128], BF16, tag="w2T")
        s2T = persist.tile([128, HC, DC, 128], BF16, tag="s2T")
        # hT holds one piece's activations [hid, piece_cols]
        hT = persist.tile([128, HC, 512], BF16, tag="hT")

        # --- PE warmup: dummy matmuls with no data deps keep the tensor
        # engine busy (and its clock ramping) while the first DMAs land.
        wup = persist.tile([128, 512], BF16, tag="wup")
        nc.vector.memset(wup[:], 0)
        wup_p = wup_ps.tile([128, 512], F32, tag="wup")
        for _ in range(_g("K_WUP", 6)):
            nc.tensor.matmul(wup_p[:], wup[:, 0:128], wup[:], start=True, stop=True)

        f_d = (f1_d, f3_d, f2_d) if F else None
        for _rep in range(reps):
            _kernel_body(nc, pieces, N, xT_d, w1_d, w3_d, w2_d, s1_d, s3_d,
                         s2_d, y_d, xT, w1T, w3T, s1T, s3T, w2T, s2T, hT,
                         silu_p, yo_p, h_ps, y_ps, persist, f_d)

    _split_multi_waits(nc)
    try:
        _CACHE["makespan_ns"] = max(e[2] for e in tc._perfetto_entries)
    except Exception:
        _CACHE["makespan_ns"] = None
    return nc


def _kernel_body(nc, pieces, N, xT_d, w1_d, w3_d, w2_d, s1_d, s3_d, s2_d,
                 y_d, xT, w1T, w3T, s1T, s3T, w2T, s2T, hT,
                 silu_p, yo_p, h_ps, y_ps, persist=None, f_d=None):
    if True:
        # --- DMA schedule: first-piece x interleaved with first w1/w3
        # hid-chunks (PE consumes chunk-by-chunk), then the rest.
        c00, c0len = pieces[0][0], pieces[0][1]
        if os.environ.get("K_PRO", "0") == "1":
            # w1 hc0 first so the first p1 series starts as x chunks stream in
            nc.sync.dma_start(w1T[:, 0], w1_d[:, 0])
            for dc in range(DC):
                nc.sync.dma_start(
                    xT[:, dc, c00:c00 + c0len], xT_d[:, dc, c00:c00 + c0len]
                )
            nc.sync.dma_start(w3T[:, 0], w3_d[:, 0])
            nc.sync.dma_start(w1T[:, 1], w1_d[:, 1])
            nc.sync.dma_start(w3T[:, 1], w3_d[:, 1])
        else:
            for dc in range(DC):
                nc.sync.dma_start(
                    xT[:, dc, c00:c00 + c0len], xT_d[:, dc, c00:c00 + c0len]
                )
                if dc < 2:
                    nc.sync.dma_start(w1T[:, dc], w1_d[:, dc])
                    nc.sync.dma_start(w3T[:, dc], w3_d[:, dc])
        for hc in range(2, HC):
            nc.sync.dma_start(w1T[:, hc], w1_d[:, hc])
            nc.sync.dma_start(w3T[:, hc], w3_d[:, hc])
            if hc == 7 and c0len < N:
                nc.sync.dma_start(xT[:, :, c0len:N], xT_d[:, :, c0len:N])
        nc.sync.dma_start(w2T[:], w2_d[:])
        nc.sync.dma_start(s1T[:], s1_d[:])
        nc.sync.dma_start(s3T[:], s3_d[:])
        nc.sync.dma_start(s2T[:], s2_d[:])
        if f_d is not None:
            # flex weights reuse the expert-weight SBUF slots (same tags,
            # bufs=1 ring): the DMAs wait for the expert pieces' last reads.
            f1T = persist.tile([128, HC, DC, 128], BF16, tag="w1T")
            f3T = persist.tile([128, HC, DC, 128], BF16, tag="w3T")
            f2T = persist.tile([128, HC, DC, 128], BF16, tag="w2T")
            for hc in range(HC):
                nc.sync.dma_start(f1T[:, hc], f_d[0][:, hc])
                nc.sync.dma_start(f3T[:, hc], f_d[1][:, hc])
            nc.sync.dma_start(f2T[:], f_d[2][:])

        n_borrow = int(os.environ.get("K_BORROW", "3"))
        psum_dma = os.environ.get("K_PSDMA", "0") == "1"
        for pi, (c0, clen, ws) in enumerate(pieces):
            if ws == 0:
                a1T, a3T, a2T = (w1T, w3T, w2T)
            elif ws == 1:
                a1T, a3T, a2T = (s1T, s3T, s2T)
            else:
                a1T, a3T, a2T = (f1T, f3T, f2T)
            csl = slice(c0, c0 + clen)
            # h = silu(w1 x) * (w3 x), written to hT[:, :, 0:clen]
            for hc in range(HC):
                # y_ps banks are idle until the first mm2 — borrow them for
                # the first few h chunks so the startup WAR chain never waits
                hp = y_ps if (pi == 0 and hc < n_borrow) else h_ps
                p1 = hp.tile([128, 512], F32, tag="hps" if hp is h_ps else "y")
                for dc in range(DC):
                    nc.tensor.matmul(
                        p1[:, 0:clen], a1T[:, hc, dc], xT[:, dc, csl],
                        start=(dc == 0), stop=(dc == DC - 1),
                    )
                p3 = hp.tile([128, 512], F32, tag="hps" if hp is h_ps else "y")
                for dc in range(DC):
                    nc.tensor.matmul(
                        p3[:, 0:clen], a3T[:, hc, dc], xT[:, dc, csl],
                        start=(dc == 0), stop=(dc == DC - 1),
                    )
                sl = silu_p.tile([128, 512], BF16, tag="silu")
                nc.scalar.activation(sl[:, 0:clen], p1[:, 0:clen], AF.Silu)
                nc.vector.tensor_tensor(
                    hT[:, hc, 0:clen], sl[:, 0:clen], p3[:, 0:clen], op=OP.mult
                )

            # y[d, t] = w2.T h for this piece (d-major output)
            for dc in range(DC):
                py = y_ps.tile([128, 512], F32, tag="y")
                for hc in range(HC):
                    nc.tensor.matmul(
                        py[:, 0:clen], a2T[:, hc, dc], hT[:, hc, 0:clen],
                        start=(hc == 0), stop=(hc == HC - 1),
                    )
                if psum_dma:
                    nc.sync.dma_start(y_d[:, dc, csl], py[:, 0:clen])
                else:
                    yo = yo_p.tile([128, 512], F32, tag="yo")
                    nc.vector.tensor_copy(yo[:, 0:clen], py[:, 0:clen])
                    nc.sync.dma_start(y_d[:, dc, csl], yo[:, 0:clen])


_CACHE = {}


def _wT_layout(w):
    """[HID, DIM] (bf16) -> DRAM layout [128, HC, DC, 128] where
    [p, hc, dc, i] = w[hc*128 + i, dc*128 + p]."""
    return np.ascontiguousarray(
        w.reshape(HC, 128, DC, 128).transpose(3, 0, 2, 1)
    )


def _w2T_layout(w):
    """[DIM, HID] (bf16) -> DRAM layout [128, HC, DC, 128] where
    [p, hc, dc, i] = w[dc*128 + i, hc*128 + p]."""
    return np.ascontiguousarray(
        w.T.reshape(HC, 128, DC, 128).transpose(1, 0, 2, 3)
    )


def _xT_layout(tok, N):
    """[N, DIM] (bf16) -> DRAM layout [128, DC, N]."""
    return np.ascontiguousarray(tok.T.reshape(DC, 128, N).transpose(1, 0, 2))


def kernel(x, gate_w, w1, w2, w3, ws1, ws2, ws3):
    x = np.asarray(x, dtype=np.float32)
    gate_w = np.asarray(gate_w, dtype=np.float32)
    w1 = np.asarray(w1, dtype=np.float32)
    w2 = np.asarray(w2, dtype=np.float32)
    w3 = np.asarray(w3, dtype=np.float32)
    ws1 = np.asarray(ws1, dtype=np.float32)
    ws2 = np.asarray(ws2, dtype=np.float32)
    ws3 = np.asarray(ws3, dtype=np.float32)

    B, S, D = x.shape
    x2 = np.ascontiguousarray(x.reshape(-1, D))
    Tn = x2.shape[0]
    assert Tn == T and D == DIM

    # --- gate: softmax + top-2 + weight normalization (host)
    logits = x2 @ gate_w.T
    m = logits.max(-1, keepdims=True)
    sm = np.exp(logits - m)
    sm /= sm.sum(-1, keepdims=True)
    ti = np.argsort(-sm, axis=-1)[:, :2]
    tw = np.take_along_axis(sm, ti, axis=-1)
    tw = tw / (tw.sum(-1, keepdims=True) + 1e-20)

    idx_e, cw_e = [], []
    for e in range(E):
        sel = (ti[:, 0] == e) | (ti[:, 1] == e)
        idx = np.nonzero(sel)[0]
        w_tok = np.where(ti[idx, 0] == e, tw[idx, 0], 0.0) + np.where(
            ti[idx, 1] == e, tw[idx, 1], 0.0
        )
        idx_e.append(idx)
        cw_e.append(w_tok.astype(np.float32))

    maxL = max(len(i) for i in idx_e)
    # A true flex segment (filling pad columns with shared work) costs a full
    # extra weight-set DMA per core and models worse (113.7us vs 105.8us);
    # kept only behind K_FLEX=1 for experiments.
    use_flex = os.environ.get("K_FLEX", "0") == "1" and maxL > 512
    if use_flex:
        A = 512
        F = -(-(maxL - A) // 8) * 8
        n_over = sum(1 for i in idx_e if len(i) > A)
        BSEG = -(-(T - (N_CORES - n_over) * F) // N_CORES // 8) * 8
    else:
        A = max(128, -(-maxL // 8) * 8)
        F = 0
        BSEG = B_SH
    N = A + F + BSEG

    key = ("nc", A, F, BSEG)
    if key not in _CACHE:
        _CACHE[key] = _build_kernel(A, F=F, B=BSEG)
    nc = _CACHE[key]
    _CACHE["nc"] = nc

    x_bf = x2.astype(bfloat16)
    sh_w = (
        _wT_layout(ws1.astype(bfloat16)),
        _wT_layout(ws3.astype(bfloat16)),
        _w2T_layout(ws2.astype(bfloat16)),
    )
    # distribute the shared-expert tokens: flex-shared cores get F tokens in
    # their flex segment, every core gets up to B in its shared segment
    sh_pos = 0
    in_maps = []
    core_meta = []
    for c in range(N_CORES):
        idx = idx_e[c]
        exp_n = min(len(idx), A)
        over_n = len(idx) - exp_n  # >0 only when use_flex and this expert overflows
        ew = (
            _wT_layout(w1[c].astype(bfloat16)),
            _wT_layout(w3[c].astype(bfloat16)),
            _w2T_layout(w2[c].astype(bfloat16)),
        )
        tok = np.zeros((N, DIM), dtype=bfloat16)
        tok[:exp_n] = x_bf[idx[:exp_n]]
        m = {
            "xT": None,
            "w1T": ew[0], "w3T": ew[1], "w2T": ew[2],
            "s1T": sh_w[0], "s3T": sh_w[1], "s2T": sh_w[2],
        }
        flex_sh_idx = None
        if F:
            if over_n:
                tok[A:A + over_n] = x_bf[idx[exp_n:]]
                m["f1T"], m["f3T"], m["f2T"] = ew
            else:
                fn = min(F, T - sh_pos)
                flex_sh_idx = np.arange(sh_pos, sh_pos + fn)
                tok[A:A + fn] = x_bf[flex_sh_idx]
                sh_pos += fn
                m["f1T"], m["f3T"], m["f2T"] = sh_w
        bn = min(BSEG, T - sh_pos)
        sh_idx = np.arange(sh_pos, sh_pos + bn)
        tok[A + F:A + F + bn] = x_bf[sh_idx]
        sh_pos += bn
        m["xT"] = _xT_layout(tok, N)
        in_maps.append(m)
        core_meta.append((exp_n, over_n, flex_sh_idx, sh_idx))
    assert sh_pos == T, f"shared token distribution bug: {sh_pos} != {T}"

    _CACHE["last_in_maps"] = in_maps
    res = run_bass_kernel_spmd(nc, in_maps, list(range(N_CORES)))

    y = np.zeros((T, DIM), dtype=np.float32)
    for c in range(N_CORES):
        yc_dm = np.asarray(res.results[c]["y"], dtype=np.float32)  # [128, DC, N]
        yc = yc_dm.transpose(1, 0, 2).reshape(DIM, N).T  # [N, DIM]
        idx = idx_e[c]
        exp_n, over_n, flex_sh_idx, sh_idx = core_meta[c]
        y[idx[:exp_n]] += cw_e[c][:exp_n, None] * yc[:exp_n]
        if over_n:
            y[idx[exp_n:]] += cw_e[c][exp_n:, None] * yc[A:A + over_n]
        elif flex_sh_idx is not None and len(flex_sh_idx):
            y[flex_sh_idx] += yc[A:A + len(flex_sh_idx)]
        y[sh_idx] += yc[A + F:A + F + len(sh_idx)]
    return y.reshape(B, S, D)


# revision 37
# speedup vs baseline: 1.0042x; 1.0042x over previous
"""Bass/Trainium2 kernel for nn_MOEFeedForward (8-expert top-2 MoE + shared expert).

Sharding: expert-parallel with host-side dispatch. The host computes the gate
(softmax + top-2) and routes tokens: core c receives expert c's tokens (padded
to capacity A = ceil8(max expert load)) plus a 1/8 token-slice of the
shared-expert work (B = 256 tokens). Every core runs A+B token-FFN columns of
identical shape (hid=2048, dim=768) — balanced, no 8x dense overcompute. The
host applies the gate weights and scatter-adds per-core outputs into the full
result.

Device kernel: all operands pre-transposed/laid out on the host so the device
does only contiguous DMAs and back-to-back bf16 matmuls at 1 col/cycle.
Column pieces of <=512 run mm1/mm3 (hid-chunked, PSUM-accumulated over the 6
dim-chunks), silu*mul drains to bf16 hT, then mm2 in d-major form
(y[d, t], 6 dim psums contracting 16 hid chunks). Dummy PE warmup matmuls
ramp the tensor-engine clock while the first DMAs land. Cost-model makespan
~105.8us/core vs ~99.8us pure-matmul floor at 2.4 GHz.

Self-contained: hardcodes shapes from the problem spec.
"""
import os
import sys

sys.path.insert(0, "/opt/trn_rl_repo")

from contextlib import ExitStack

import numpy as np
from ml_dtypes import bfloat16

import concourse.bass as bass
import concourse.tile as tile
from concourse import mybir
from concourse.bass_utils import run_bass_kernel_spmd
from concourse.vector_clock import ScopedClock

DIM = 768
HID = 2048
E = 8
T = 2048
N_CORES = 8
B_SH = T // N_CORES  # shared-expert tokens per core (256)
DC = DIM // 128      # 6 d-chunks
HC = HID // 128      # 16 hid-chunks

F32 = mybir.dt.float32
BF16 = mybir.dt.bfloat16

AF = mybir.ActivationFunctionType
OP = mybir.AluOpType


# ---------------------------------------------------------------------------
# Walrus in this container rejects CTRL instructions (NoOp/Drain) carrying
# more than one sem wait. TileContext's tail drain carries one wait per
# outstanding semaphore. Replace it with a chain of SP nops (one wait each)
# followed by a bare drain.
def _patched_drain_and_barrier(self, tick_clock, wait_clock):
    import bass_rust

    nop_inst = self.nc.sync.nop(nofuse=True, hint="pre_drain_wait_funnel")
    wait_clock.add_sem_waits(
        nop_inst.ins, ScopedClock({None: tick_clock.global_clock})
    )
    si = nop_inst.ins.sync_info
    waits = list(si.on_wait) if si else []
    if len(waits) > 1:
        nop_inst.ins.sync_info.on_wait = waits[:1]
        for w in waits[1:]:
            extra = self.nc.sync.nop(nofuse=True, hint="pre_drain_wait_funnel")
            extra.ins.sync_info = bass_rust.SyncInfo(on_wait=[w], on_update=[])
    self.nc.sync.drain()

    self.nc.all_engine_barrier()
    assert self.sems is not None
    popped = self.nc._tile_sem_poison_stack.pop()
    assert popped is self._sem_poison
    self.nc.clear_and_free_semaphores(list(self.sems.allocated().values()))
    self.nc.all_engine_barrier()


tile.TileContext._drain_and_barrier = _patched_drain_and_barrier


def _split_multi_waits(nc, max_waits=1):
    """This walrus build allows at most one sem wait per instruction. Hoist
    extra waits onto same-engine nops inserted immediately before."""
    import bass_rust

    n_split = 0
    for f in nc.m.functions:
        for bb in f.blocks:
            il = bb.instructions
            i = 0
            while i < len(il):
                inst = il[i]
                si = inst.sync_info
                if si is None or len(si.on_wait) <= max_waits:
                    i += 1
                    continue
                waits = list(si.on_wait)
                si.on_wait = waits[:max_waits]
                for k, w in enumerate(waits[max_waits:]):
                    nop = mybir.InstNoOp(
                        name=f"{inst.name}-wsplit{k}", ins=[], outs=[]
                    )
                    nop.engine = inst.engine
                    nop.sync_info = bass_rust.SyncInfo(on_wait=[w], on_update=[])
                    il.insert(i, nop)
                    i += 1
                n_split += 1
                i += 1
    return n_split
# ---------------------------------------------------------------------------


def _build_kernel(A, reps=1, F=0, B=B_SH):
    """A: expert-token capacity. Columns [0, A) use the expert weight set;
    with F>0, columns [A, A+F) use a per-core 'flex' weight set (host fills
    with either this core's expert weights or the shared weights); columns
    [A+F, A+F+B) use the shared weight set.
    Output y is d-major: y_d[p, dc, t] = y[t, dc*128+p].
    reps>1 repeats the whole compute (for benchmarking)."""
    N = A + F + B
    nc = bass.Bass()
    xT_d = nc.dram_tensor("xT", [128, DC, N], BF16, kind="ExternalInput")
    w1_d = nc.dram_tensor("w1T", [128, HC, DC, 128], BF16, kind="ExternalInput")
    w3_d = nc.dram_tensor("w3T", [128, HC, DC, 128], BF16, kind="ExternalInput")
    w2_d = nc.dram_tensor("w2T", [128, HC, DC, 128], BF16, kind="ExternalInput")
    s1_d = nc.dram_tensor("s1T", [128, HC, DC, 128], BF16, kind="ExternalInput")
    s3_d = nc.dram_tensor("s3T", [128, HC, DC, 128], BF16, kind="ExternalInput")
    s2_d = nc.dram_tensor("s2T", [128, HC, DC, 128], BF16, kind="ExternalInput")
    if F:
        f1_d = nc.dram_tensor("f1T", [128, HC, DC, 128], BF16, kind="ExternalInput")
        f3_d = nc.dram_tensor("f3T", [128, HC, DC, 128], BF16, kind="ExternalInput")
        f2_d = nc.dram_tensor("f2T", [128, HC, DC, 128], BF16, kind="ExternalInput")
    y_d = nc.dram_tensor("y", [128, DC, N], F32, kind="ExternalOutput")

    # column pieces: (start, len, weight-set)
    first = int(os.environ.get("K_P0", "512"))
    exp_pieces = []
    c0 = 0
    while c0 < A:
        ln = min(first if c0 == 0 else 512, A - c0)
        exp_pieces.append((c0, ln, 0))
        c0 += ln
    # flex is processed LAST: its weights reuse w1/w3/w2's SBUF (same pool
    # tags), so their DMA can only land after the expert pieces finish.
    pieces = exp_pieces + [(A + F, B, 1)]
    if F:
        pieces = pieces + [(A, F, 2)]

    _g = lambda k, d: int(os.environ.get(k, str(d)))
    with tile.TileContext(nc) as tc, ExitStack() as ctx:
        persist = ctx.enter_context(tc.tile_pool(name="persist", bufs=1))
        silu_p = ctx.enter_context(tc.tile_pool(name="silu", bufs=_g("K_SILU", 3)))
        yo_p = ctx.enter_context(tc.tile_pool(name="yo", bufs=_g("K_YO", 3)))
        h_ps = ctx.enter_context(
            tc.tile_pool(name="h_ps", bufs=_g("K_HPS", 4), space="PSUM"))
        y_ps = ctx.enter_context(
            tc.tile_pool(name="y_ps", bufs=_g("K_YPS", 3), space="PSUM"))
        wup_ps = ctx.enter_context(tc.tile_pool(name="wup_ps", bufs=1, space="PSUM"))

        xT = persist.tile([128, DC, N], BF16, tag="xT")
        w1T = persist.tile([128, HC, DC, 128], BF16, tag="w1T")
        w3T = persist.tile([128, HC, DC, 128], BF16, tag="w3T")
        s1T = persist.tile([128, HC, DC, 128], BF16, tag="s1T")
        s3T = persist.tile([128, HC, DC, 128], BF16, tag="s3T")
        w2T = persist.tile([128, HC, DC, 128], BF16, tag="w2T")
        s2T = persist.tile([128, HC, DC, 128], BF16, tag="s2T")
        # hT holds one piece's activations [hid, piece_cols]
        hT = persist.tile([128, HC, 512], BF16, tag="hT")

        # --- PE warmup: dummy matmuls with no data deps keep the tensor
        # engine busy (and its clock ramping) while the first DMAs land.
        wup = persist.tile([128, 512], BF16, tag="wup")
        nc.vector.memset(wup[:], 0)
        wup_p = wup_ps.tile([128, 512], F32, tag="wup")
        for _ in range(_g("K_WUP", 6)):
            nc.tensor.matmul(wup_p[:], wup[:, 0:128], wup[:], start=True, stop=True)

        f_d = (f1_d, f3_d, f2_d) if F else None
        for _rep in range(reps):
            _kernel_body(nc, pieces, N, xT_d, w1_d, w3_d, w2_d, s1_d, s3_d,
                         s2_d, y_d, xT, w1T, w3T, s1T, s3T, w2T, s2T, hT,
                         silu_p, yo_p, h_ps, y_ps, persist, f_d)

    _split_multi_waits(nc)
    try:
        _CACHE["makespan_ns"] = max(e[2] for e in tc._perfetto_entries)
    except Exception:
        _CACHE["makespan_ns"] = None
    return nc


def _kernel_body(nc, pieces, N, xT_d, w1_d, w3_d, w2_d, s1_d, s3_d, s2_d,
                 y_d, xT, w1T, w3T, s1T, s3T, w2T, s2T, hT,
                 silu_p, yo_p, h_ps, y_ps, persist=None, f_d=None):
    if True:
        # --- DMA schedule: first-piece x interleaved with first w1/w3
        # hid-chunks (PE consumes chunk-by-chunk), then the rest.
        c00, c0len = pieces[0][0], pieces[0][1]
        if os.environ.get("K_PRO", "0") == "1":
            # w1 hc0 first so the first p1 series starts as x chunks stream in
            nc.sync.dma_start(w1T[:, 0], w1_d[:, 0])
            for dc in range(DC):
                nc.sync.dma_start(
                    xT[:, dc, c00:c00 + c0len], xT_d[:, dc, c00:c00 + c0len]
                )
            nc.sync.dma_start(w3T[:, 0], w3_d[:, 0])
            nc.sync.dma_start(w1T[:, 1], w1_d[:, 1])
            nc.sync.dma_start(w3T[:, 1], w3_d[:, 1])
        else:
            for dc in range(DC):
                nc.sync.dma_start(
                    xT[:, dc, c00:c00 + c0len], xT_d[:, dc, c00:c00 + c0len]
                )
                if dc < 2:
                    nc.sync.dma_start(w1T[:, dc], w1_d[:, dc])
                    nc.sync.dma_start(w3T[:, dc], w3_d[:, dc])
        for hc in range(2, HC):
            nc.sync.dma_start(w1T[:, hc], w1_d[:, hc])
            nc.sync.dma_start(w3T[:, hc], w3_d[:, hc])
            if hc == 7 and c0len < N:
                nc.sync.dma_start(xT[:, :, c0len:N], xT_d[:, :, c0len:N])
        nc.sync.dma_start(w2T[:], w2_d[:])
        nc.sync.dma_start(s1T[:], s1_d[:])
        nc.sync.dma_start(s3T[:], s3_d[:])
        nc.sync.dma_start(s2T[:], s2_d[:])
        if f_d is not None:
            # flex weights reuse the expert-weight SBUF slots (same tags,
            # bufs=1 ring): the DMAs wait for the expert pieces' last reads.
            f1T = persist.tile([128, HC, DC, 128], BF16, tag="w1T")
            f3T = persist.tile([128, HC, DC, 128], BF16, tag="w3T")
            f2T = persist.tile([128, HC, DC, 128], BF16, tag="w2T")
            for hc in range(HC):
                nc.sync.dma_start(f1T[:, hc], f_d[0][:, hc])
                nc.sync.dma_start(f3T[:, hc], f_d[1][:, hc])
            nc.sync.dma_start(f2T[:], f_d[2][:])

        n_borrow = int(os.environ.get("K_BORROW", "0"))
        psum_dma = os.environ.get("K_PSDMA", "0") == "1"
        for pi, (c0, clen, ws) in enumerate(pieces):
            if ws == 0:
                a1T, a3T, a2T = (w1T, w3T, w2T)
            elif ws == 1:
                a1T, a3T, a2T = (s1T, s3T, s2T)
            else:
                a1T, a3T, a2T = (f1T, f3T, f2T)
            csl = slice(c0, c0 + clen)
            # h = silu(w1 x) * (w3 x), written to hT[:, :, 0:clen]
            for hc in range(HC):
                # y_ps banks are idle until the first mm2 — borrow them for
                # the first few h chunks so the startup WAR chain never waits
                hp = y_ps if (pi == 0 and hc < n_borrow) else h_ps
                p1 = hp.tile([128, 512], F32, tag="hps" if hp is h_ps else "y")
                for dc in range(DC):
                    nc.tensor.matmul(
                        p1[:, 0:clen], a1T[:, hc, dc], xT[:, dc, csl],
                        start=(dc == 0), stop=(dc == DC - 1),
                    )
                p3 = hp.tile([128, 512], F32, tag="hps" if hp is h_ps else "y")
                for dc in range(DC):
                    nc.tensor.matmul(
                        p3[:, 0:clen], a3T[:, hc, dc], xT[:, dc, csl],
                        start=(dc == 0), stop=(dc == DC - 1),
                    )
                sl = silu_p.tile([128, 512], BF16, tag="silu")
                nc.scalar.activation(sl[:, 0:clen], p1[:, 0:clen], AF.Silu)
                nc.vector.tensor_tensor(
                    hT[:, hc, 0:clen], sl[:, 0:clen], p3[:, 0:clen], op=OP.mult
                )

            # y[d, t] = w2.T h for this piece (d-major output)
            for dc in range(DC):
                py = y_ps.tile([128, 512], F32, tag="y")
                for hc in range(HC):
                    nc.tensor.matmul(
                        py[:, 0:clen], a2T[:, hc, dc], hT[:, hc, 0:clen],
                        start=(hc == 0), stop=(hc == HC - 1),
                    )
                if psum_dma:
                    nc.sync.dma_start(y_d[:, dc, csl], py[:, 0:clen])
                else:
                    yo = yo_p.tile([128, 512], F32, tag="yo")
                    nc.vector.tensor_copy(yo[:, 0:clen], py[:, 0:clen])
                    nc.sync.dma_start(y_d[:, dc, csl], yo[:, 0:clen])


_CACHE = {}


def _wT_layout(w):
    """[HID, DIM] (bf16) -> DRAM layout [128, HC, DC, 128] where
    [p, hc, dc, i] = w[hc*128 + i, dc*128 + p]."""
    return np.ascontiguousarray(
        w.reshape(HC, 128, DC, 128).transpose(3, 0, 2, 1)
    )


def _w2T_layout(w):
    """[DIM, HID] (bf16) -> DRAM layout [128, HC, DC, 128] where
    [p, hc, dc, i] = w[dc*128 + i, hc*128 + p]."""
    return np.ascontiguousarray(
        w.T.reshape(HC, 128, DC, 128).transpose(1, 0, 2, 3)
    )


def _xT_layout(tok, N):
    """[N, DIM] (bf16) -> DRAM layout [128, DC, N]."""
    return np.ascontiguousarray(tok.T.reshape(DC, 128, N).transpose(1, 0, 2))


def kernel(x, gate_w, w1, w2, w3, ws1, ws2, ws3):
    x = np.asarray(x, dtype=np.float32)
    gate_w = np.asarray(gate_w, dtype=np.float32)
    w1 = np.asarray(w1, dtype=np.float32)
    w2 = np.asarray(w2, dtype=np.float32)
    w3 = np.asarray(w3, dtype=np.float32)
    ws1 = np.asarray(ws1, dtype=np.float32)
    ws2 = np.asarray(ws2, dtype=np.float32)
    ws3 = np.asarray(ws3, dtype=np.float32)

    B, S, D = x.shape
    x2 = np.ascontiguousarray(x.reshape(-1, D))
    Tn = x2.shape[0]
    assert Tn == T and D == DIM

    # --- gate: softmax + top-2 + weight normalization (host)
    logits = x2 @ gate_w.T
    m = logits.max(-1, keepdims=True)
    sm = np.exp(logits - m)
    sm /= sm.sum(-1, keepdims=True)
    ti = np.argsort(-sm, axis=-1)[:, :2]
    tw = np.take_along_axis(sm, ti, axis=-1)
    tw = tw / (tw.sum(-1, keepdims=True) + 1e-20)

    idx_e, cw_e = [], []
    for e in range(E):
        sel = (ti[:, 0] == e) | (ti[:, 1] == e)
        idx = np.nonzero(sel)[0]
        w_tok = np.where(ti[idx, 0] == e, tw[idx, 0], 0.0) + np.where(
            ti[idx, 1] == e, tw[idx, 1], 0.0
        )
        idx_e.append(idx)
        cw_e.append(w_tok.astype(np.float32))

    maxL = max(len(i) for i in idx_e)
    # A true flex segment (filling pad columns with shared work) costs a full
    # extra weight-set DMA per core and models worse (113.7us vs 105.8us);
    # kept only behind K_FLEX=1 for experiments.
    use_flex = os.environ.get("K_FLEX", "0") == "1" and maxL > 512
    if use_flex:
        A = 512
        F = -(-(maxL - A) // 8) * 8
        n_over = sum(1 for i in idx_e if len(i) > A)
        BSEG = -(-(T - (N_CORES - n_over) * F) // N_CORES // 8) * 8
    else:
        A = max(128, -(-maxL // 8) * 8)
        F = 0
        BSEG = B_SH
    N = A + F + BSEG

    key = ("nc", A, F, BSEG)
    if key not in _CACHE:
        _CACHE[key] = _build_kernel(A, F=F, B=BSEG)
    nc = _CACHE[key]
    _CACHE["nc"] = nc

    x_bf = x2.astype(bfloat16)
    sh_w = (
        _wT_layout(ws1.astype(bfloat16)),
        _wT_layout(ws3.astype(bfloat16)),
        _w2T_layout(ws2.astype(bfloat16)),
    )
    # distribute the shared-expert tokens: flex-shared cores get F tokens in
    # their flex segment, every core gets up to B in its shared segment
    sh_pos = 0
    in_maps = []
    core_meta = []
    for c in range(N_CORES):
        idx = idx_e[c]
        exp_n = min(len(idx), A)
        over_n = len(idx) - exp_n  # >0 only when use_flex and this expert overflows
        ew = (
            _wT_layout(w1[c].astype(bfloat16)),
            _wT_layout(w3[c].astype(bfloat16)),
            _w2T_layout(w2[c].astype(bfloat16)),
        )
        tok = np.zeros((N, DIM), dtype=bfloat16)
        tok[:exp_n] = x_bf[idx[:exp_n]]
        m = {
            "xT": None,
            "w1T": ew[0], "w3T": ew[1], "w2T": ew[2],
            "s1T": sh_w[0], "s3T": sh_w[1], "s2T": sh_w[2],
        }
        flex_sh_idx = None
        if F:
            if over_n:
                tok[A:A + over_n] = x_bf[idx[exp_n:]]
                m["f1T"], m["f3T"], m["f2T"] = ew
            else:
                fn = min(F, T - sh_pos)
                flex_sh_idx = np.arange(sh_pos, sh_pos + fn)
                tok[A:A + fn] = x_bf[flex_sh_idx]
                sh_pos += fn
                m["f1T"], m["f3T"], m["f2T"] = sh_w
        bn = min(BSEG, T - sh_pos)
        sh_idx = np.arange(sh_pos, sh_pos + bn)
        tok[A + F:A + F + bn] = x_bf[sh_idx]
        sh_pos += bn
        m["xT"] = _xT_layout(tok, N)
        in_maps.append(m)
        core_meta.append((exp_n, over_n, flex_sh_idx, sh_idx))
    assert sh_pos == T, f"shared token distribution bug: {sh_pos} != {T}"

    _CACHE["last_in_maps"] = in_maps
    res = run_bass_kernel_spmd(nc, in_maps, list(range(N_CORES)))

    y = np.zeros((T, DIM), dtype=np.float32)
    for c in range(N_CORES):
        yc_dm = np.asarray(res.results[c]["y"], dtype=np.float32)  # [128, DC, N]
        yc = yc_dm.transpose(1, 0, 2).reshape(DIM, N).T  # [N, DIM]
        idx = idx_e[c]
        exp_n, over_n, flex_sh_idx, sh_idx = core_meta[c]
        y[idx[:exp_n]] += cw_e[c][:exp_n, None] * yc[:exp_n]
        if over_n:
            y[idx[exp_n:]] += cw_e[c][exp_n:, None] * yc[A:A + over_n]
        elif flex_sh_idx is not None and len(flex_sh_idx):
            y[flex_sh_idx] += yc[A:A + len(flex_sh_idx)]
        y[sh_idx] += yc[A + F:A + F + len(sh_idx)]
    return y.reshape(B, S, D)


# revision 41
# speedup vs baseline: 1.0054x; 1.0012x over previous
"""Bass/Trainium2 kernel for nn_MOEFeedForward (8-expert top-2 MoE + shared expert).

Sharding: expert-parallel with host-side dispatch. The host computes the gate
(softmax + top-2) and routes tokens: core c receives expert c's tokens (padded
to capacity A = ceil8(max expert load)) plus a 1/8 token-slice of the
shared-expert work (B = 256 tokens). Every core runs A+B token-FFN columns of
identical shape (hid=2048, dim=768) — balanced, no 8x dense overcompute. The
host applies the gate weights and scatter-adds per-core outputs into the full
result.

Device kernel: all operands pre-transposed/laid out on the host so the device
does only contiguous DMAs and back-to-back bf16 matmuls at 1 col/cycle.
Column pieces of <=512 run mm1/mm3 (hid-chunked, PSUM-accumulated over the 6
dim-chunks), silu*mul drains to bf16 hT, then mm2 in d-major form
(y[d, t], 6 dim psums contracting 16 hid chunks). Dummy PE warmup matmuls
ramp the tensor-engine clock while the first DMAs land. Cost-model makespan
~105.8us/core vs ~99.8us pure-matmul floor at 2.4 GHz.

Self-contained: hardcodes shapes from the problem spec.
"""
import os
import sys

sys.path.insert(0, "/opt/trn_rl_repo")

from contextlib import ExitStack

import numpy as np
from ml_dtypes import bfloat16

import concourse.bass as bass
import concourse.tile as tile
from concourse import mybir
from concourse.bass_utils import run_bass_kernel_spmd
from concourse.vector_clock import ScopedClock

DIM = 768
HID = 2048
E = 8
T = 2048
N_CORES = 8
B_SH = T // N_CORES  # shared-expert tokens per core (256)
DC = DIM // 128      # 6 d-chunks
HC = HID // 128      # 16 hid-chunks

F32 = mybir.dt.float32
BF16 = mybir.dt.bfloat16

AF = mybir.ActivationFunctionType
OP = mybir.AluOpType


# ---------------------------------------------------------------------------
# Walrus in this container rejects CTRL instructions (NoOp/Drain) carrying
# more than one sem wait. TileContext's tail drain carries one wait per
# outstanding semaphore. Replace it with a chain of SP nops (one wait each)
# followed by a bare drain.
def _patched_drain_and_barrier(self, tick_clock, wait_clock):
    import bass_rust

    nop_inst = self.nc.sync.nop(nofuse=True, hint="pre_drain_wait_funnel")
    wait_clock.add_sem_waits(
        nop_inst.ins, ScopedClock({None: tick_clock.global_clock})
    )
    si = nop_inst.ins.sync_info
    waits = list(si.on_wait) if si else []
    if len(waits) > 1:
        nop_inst.ins.sync_info.on_wait = waits[:1]
        for w in waits[1:]:
            extra = self.nc.sync.nop(nofuse=True, hint="pre_drain_wait_funnel")
            extra.ins.sync_info = bass_rust.SyncInfo(on_wait=[w], on_update=[])
    self.nc.sync.drain()

    self.nc.all_engine_barrier()
    assert self.sems is not None
    popped = self.nc._tile_sem_poison_stack.pop()
    assert popped is self._sem_poison
    self.nc.clear_and_free_semaphores(list(self.sems.allocated().values()))
    self.nc.all_engine_barrier()


tile.TileContext._drain_and_barrier = _patched_drain_and_barrier


def _split_multi_waits(nc, max_waits=1):
    """This walrus build allows at most one sem wait per instruction. Hoist
    extra waits onto same-engine nops inserted immediately before."""
    import bass_rust

    n_split = 0
    for f in nc.m.functions:
        for bb in f.blocks:
            il = bb.instructions
            i = 0
            while i < len(il):
                inst = il[i]
                si = inst.sync_info
                if si is None or len(si.on_wait) <= max_waits:
                    i += 1
                    continue
                waits = list(si.on_wait)
                si.on_wait = waits[:max_waits]
                for k, w in enumerate(waits[max_waits:]):
                    nop = mybir.InstNoOp(
                        name=f"{inst.name}-wsplit{k}", ins=[], outs=[]
                    )
                    nop.engine = inst.engine
                    nop.sync_info = bass_rust.SyncInfo(on_wait=[w], on_update=[])
                    il.insert(i, nop)
                    i += 1
                n_split += 1
                i += 1
    return n_split
# ---------------------------------------------------------------------------


def _build_kernel(A, reps=1, F=0, B=B_SH):
    """A: expert-token capacity. Columns [0, A) use the expert weight set;
    with F>0, columns [A, A+F) use a per-core 'flex' weight set (host fills
    with either this core's expert weights or the shared weights); columns
    [A+F, A+F+B) use the shared weight set.
    Output y is d-major: y_d[p, dc, t] = y[t, dc*128+p].
    reps>1 repeats the whole compute (for benchmarking)."""
    N = A + F + B
    nc = bass.Bass()
    xT_d = nc.dram_tensor("xT", [128, DC, N], BF16, kind="ExternalInput")
    w1_d = nc.dram_tensor("w1T", [128, HC, DC, 128], BF16, kind="ExternalInput")
    w3_d = nc.dram_tensor("w3T", [128, HC, DC, 128], BF16, kind="ExternalInput")
    w2_d = nc.dram_tensor("w2T", [128, HC, DC, 128], BF16, kind="ExternalInput")
    s1_d = nc.dram_tensor("s1T", [128, HC, DC, 128], BF16, kind="ExternalInput")
    s3_d = nc.dram_tensor("s3T", [128, HC, DC, 128], BF16, kind="ExternalInput")
    s2_d = nc.dram_tensor("s2T", [128, HC, DC, 128], BF16, kind="ExternalInput")
    if F:
        f1_d = nc.dram_tensor("f1T", [128, HC, DC, 128], BF16, kind="ExternalInput")
        f3_d = nc.dram_tensor("f3T", [128, HC, DC, 128], BF16, kind="ExternalInput")
        f2_d = nc.dram_tensor("f2T", [128, HC, DC, 128], BF16, kind="ExternalInput")
    y_d = nc.dram_tensor("y", [128, DC, N], F32, kind="ExternalOutput")

    # column pieces: (start, len, weight-set)
    first = int(os.environ.get("K_P0", "512"))
    exp_pieces = []
    c0 = 0
    while c0 < A:
        ln = min(first if c0 == 0 else 512, A - c0)
        exp_pieces.append((c0, ln, 0))
        c0 += ln
    # flex is processed LAST: its weights reuse w1/w3/w2's SBUF (same pool
    # tags), so their DMA can only land after the expert pieces finish.
    pieces = exp_pieces + [(A + F, B, 1)]
    if F:
        pieces = pieces + [(A, F, 2)]

    _g = lambda k, d: int(os.environ.get(k, str(d)))
    with tile.TileContext(nc) as tc, ExitStack() as ctx:
        persist = ctx.enter_context(tc.tile_pool(name="persist", bufs=1))
        silu_p = ctx.enter_context(tc.tile_pool(name="silu", bufs=_g("K_SILU", 3)))
        yo_p = ctx.enter_context(tc.tile_pool(name="yo", bufs=_g("K_YO", 3)))
        h_ps = ctx.enter_context(
            tc.tile_pool(name="h_ps", bufs=_g("K_HPS", 4), space="PSUM"))
        y_ps = ctx.enter_context(
            tc.tile_pool(name="y_ps", bufs=_g("K_YPS", 3), space="PSUM"))
        wup_ps = ctx.enter_context(tc.tile_pool(name="wup_ps", bufs=1, space="PSUM"))

        xT = persist.tile([128, DC, N], BF16, tag="xT")
        w1T = persist.tile([128, HC, DC, 128], BF16, tag="w1T")
        w3T = persist.tile([128, HC, DC, 128], BF16, tag="w3T")
        s1T = persist.tile([128, HC, DC, 128], BF16, tag="s1T")
        s3T = persist.tile([128, HC, DC, 128], BF16, tag="s3T")
        w2T = persist.tile([128, HC, DC, 128], BF16, tag="w2T")
        s2T = persist.tile([128, HC, DC, 128], BF16, tag="s2T")
        # hT holds one piece's activations [hid, piece_cols]
        hT = persist.tile([128, HC, 512], BF16, tag="hT")

        # --- PE warmup: dummy matmuls with no data deps keep the tensor
        # engine busy (and its clock ramping) while the first DMAs land.
        wup = persist.tile([128, 512], BF16, tag="wup")
        nc.vector.memset(wup[:], 0)
        wup_p = wup_ps.tile([128, 512], F32, tag="wup")
        for _ in range(_g("K_WUP", 6)):
            nc.tensor.matmul(wup_p[:], wup[:, 0:128], wup[:], start=True, stop=True)

        f_d = (f1_d, f3_d, f2_d) if F else None
        for _rep in range(reps):
            _kernel_body(nc, pieces, N, xT_d, w1_d, w3_d, w2_d, s1_d, s3_d,
                         s2_d, y_d, xT, w1T, w3T, s1T, s3T, w2T, s2T, hT,
                         silu_p, yo_p, h_ps, y_ps, persist, f_d, wup_ps)

    _split_multi_waits(nc)
    try:
        _CACHE["makespan_ns"] = max(e[2] for e in tc._perfetto_entries)
    except Exception:
        _CACHE["makespan_ns"] = None
    return nc


def _kernel_body(nc, pieces, N, xT_d, w1_d, w3_d, w2_d, s1_d, s3_d, s2_d,
                 y_d, xT, w1T, w3T, s1T, s3T, w2T, s2T, hT,
                 silu_p, yo_p, h_ps, y_ps, persist=None, f_d=None, wup_ps=None):
    if True:
        # --- DMA schedule: first-piece x interleaved with first w1/w3
        # hid-chunks (PE consumes chunk-by-chunk), then the rest.
        c00, c0len = pieces[0][0], pieces[0][1]
        if os.environ.get("K_PRO", "0") == "1":
            # w1 hc0 first so the first p1 series starts as x chunks stream in
            nc.sync.dma_start(w1T[:, 0], w1_d[:, 0])
            for dc in range(DC):
                nc.sync.dma_start(
                    xT[:, dc, c00:c00 + c0len], xT_d[:, dc, c00:c00 + c0len]
                )
            nc.sync.dma_start(w3T[:, 0], w3_d[:, 0])
            nc.sync.dma_start(w1T[:, 1], w1_d[:, 1])
            nc.sync.dma_start(w3T[:, 1], w3_d[:, 1])
        else:
            for dc in range(DC):
                nc.sync.dma_start(
                    xT[:, dc, c00:c00 + c0len], xT_d[:, dc, c00:c00 + c0len]
                )
                if dc < 2:
                    nc.sync.dma_start(w1T[:, dc], w1_d[:, dc])
                    nc.sync.dma_start(w3T[:, dc], w3_d[:, dc])
        for hc in range(2, HC):
            nc.sync.dma_start(w1T[:, hc], w1_d[:, hc])
            nc.sync.dma_start(w3T[:, hc], w3_d[:, hc])
            if hc == 7 and c0len < N:
                nc.sync.dma_start(xT[:, :, c0len:N], xT_d[:, :, c0len:N])
        nc.sync.dma_start(w2T[:], w2_d[:])
        nc.sync.dma_start(s1T[:], s1_d[:])
        nc.sync.dma_start(s3T[:], s3_d[:])
        nc.sync.dma_start(s2T[:], s2_d[:])
        if f_d is not None:
            # flex weights reuse the expert-weight SBUF slots (same tags,
            # bufs=1 ring): the DMAs wait for the expert pieces' last reads.
            f1T = persist.tile([128, HC, DC, 128], BF16, tag="w1T")
            f3T = persist.tile([128, HC, DC, 128], BF16, tag="w3T")
            f2T = persist.tile([128, HC, DC, 128], BF16, tag="w2T")
            for hc in range(HC):
                nc.sync.dma_start(f1T[:, hc], f_d[0][:, hc])
                nc.sync.dma_start(f3T[:, hc], f_d[1][:, hc])
            nc.sync.dma_start(f2T[:], f_d[2][:])

        n_borrow = int(os.environ.get("K_BORROW", "0"))
        psum_dma = os.environ.get("K_PSDMA", "0") == "1"
        for pi, (c0, clen, ws) in enumerate(pieces):
            if ws == 0:
                a1T, a3T, a2T = (w1T, w3T, w2T)
            elif ws == 1:
                a1T, a3T, a2T = (s1T, s3T, s2T)
            else:
                a1T, a3T, a2T = (f1T, f3T, f2T)
            csl = slice(c0, c0 + clen)
            # h = silu(w1 x) * (w3 x), written to hT[:, :, 0:clen]
            for hc in range(HC):
                # y_ps banks are idle until the first mm2 — borrow them for
                # the first few h chunks so the startup WAR chain never waits
                hp = y_ps if (pi == 0 and hc < n_borrow) else h_ps
                p1 = hp.tile([128, 512], F32, tag="hps" if hp is h_ps else "y")
                for dc in range(DC):
                    nc.tensor.matmul(
                        p1[:, 0:clen], a1T[:, hc, dc], xT[:, dc, csl],
                        start=(dc == 0), stop=(dc == DC - 1),
                    )
                p3 = hp.tile([128, 512], F32, tag="hps" if hp is h_ps else "y")
                for dc in range(DC):
                    nc.tensor.matmul(
                        p3[:, 0:clen], a3T[:, hc, dc], xT[:, dc, csl],
                        start=(dc == 0), stop=(dc == DC - 1),
                    )
                sl = silu_p.tile([128, 512], BF16, tag="silu")
                nc.scalar.activation(sl[:, 0:clen], p1[:, 0:clen], AF.Silu)
                nc.vector.tensor_tensor(
                    hT[:, hc, 0:clen], sl[:, 0:clen], p3[:, 0:clen], op=OP.mult
                )

            # y[d, t] = w2.T h for this piece (d-major output)
            for dc in range(DC):
                if os.environ.get("K_WUPY", "1") == "1" and dc % 4 == 3:
                    py = wup_ps.tile([128, 512], F32, tag="wup")
                else:
                    py = y_ps.tile([128, 512], F32, tag="y")
                for hc in range(HC):
                    nc.tensor.matmul(
                        py[:, 0:clen], a2T[:, hc, dc], hT[:, hc, 0:clen],
                        start=(hc == 0), stop=(hc == HC - 1),
                    )
                if psum_dma:
                    nc.sync.dma_start(y_d[:, dc, csl], py[:, 0:clen])
                else:
                    yo = yo_p.tile([128, 512], F32, tag="yo")
                    nc.vector.tensor_copy(yo[:, 0:clen], py[:, 0:clen])
                    nc.sync.dma_start(y_d[:, dc, csl], yo[:, 0:clen])


_CACHE = {}


def _wT_layout(w):
    """[HID, DIM] (bf16) -> DRAM layout [128, HC, DC, 128] where
    [p, hc, dc, i] = w[hc*128 + i, dc*128 + p]."""
    return np.ascontiguousarray(
        w.reshape(HC, 128, DC, 128).transpose(3, 0, 2, 1)
    )


def _w2T_layout(w):
    """[DIM, HID] (bf16) -> DRAM layout [128, HC, DC, 128] where
    [p, hc, dc, i] = w[dc*128 + i, hc*128 + p]."""
    return np.ascontiguousarray(
        w.T.reshape(HC, 128, DC, 128).transpose(1, 0, 2, 3)
    )


def _xT_layout(tok, N):
    """[N, DIM] (bf16) -> DRAM layout [128, DC, N]."""
    return np.ascontiguousarray(tok.T.reshape(DC, 128, N).transpose(1, 0, 2))


def kernel(x, gate_w, w1, w2, w3, ws1, ws2, ws3):
    x = np.asarray(x, dtype=np.float32)
    gate_w = np.asarray(gate_w, dtype=np.float32)
    w1 = np.asarray(w1, dtype=np.float32)
    w2 = np.asarray(w2, dtype=np.float32)
    w3 = np.asarray(w3, dtype=np.float32)
    ws1 = np.asarray(ws1, dtype=np.float32)
    ws2 = np.asarray(ws2, dtype=np.float32)
    ws3 = np.asarray(ws3, dtype=np.float32)

    B, S, D = x.shape
    x2 = np.ascontiguousarray(x.reshape(-1, D))
    Tn = x2.shape[0]
    assert Tn == T and D == DIM

    # --- gate: softmax + top-2 + weight normalization (host)
    logits = x2 @ gate_w.T
    m = logits.max(-1, keepdims=True)
    sm = np.exp(logits - m)
    sm /= sm.sum(-1, keepdims=True)
    ti = np.argsort(-sm, axis=-1)[:, :2]
    tw = np.take_along_axis(sm, ti, axis=-1)
    tw = tw / (tw.sum(-1, keepdims=True) + 1e-20)

    idx_e, cw_e = [], []
    for e in range(E):
        sel = (ti[:, 0] == e) | (ti[:, 1] == e)
        idx = np.nonzero(sel)[0]
        w_tok = np.where(ti[idx, 0] == e, tw[idx, 0], 0.0) + np.where(
            ti[idx, 1] == e, tw[idx, 1], 0.0
        )
        idx_e.append(idx)
        cw_e.append(w_tok.astype(np.float32))

    maxL = max(len(i) for i in idx_e)
    # A true flex segment (filling pad columns with shared work) costs a full
    # extra weight-set DMA per core and models worse (113.7us vs 105.8us);
    # kept only behind K_FLEX=1 for experiments.
    use_flex = os.environ.get("K_FLEX", "0") == "1" and maxL > 512
    if use_flex:
        A = 512
        F = -(-(maxL - A) // 8) * 8
        n_over = sum(1 for i in idx_e if len(i) > A)
        BSEG = -(-(T - (N_CORES - n_over) * F) // N_CORES // 8) * 8
    else:
        A = max(128, -(-maxL // 8) * 8)
        F = 0
        BSEG = B_SH
    N = A + F + BSEG

    key = ("nc", A, F, BSEG)
    if key not in _CACHE:
        _CACHE[key] = _build_kernel(A, F=F, B=BSEG)
    nc = _CACHE[key]
    _CACHE["nc"] = nc

    x_bf = x2.astype(bfloat16)
    sh_w = (
        _wT_layout(ws1.astype(bfloat16)),
        _wT_layout(ws3.astype(bfloat16)),
        _w2T_layout(ws2.astype(bfloat16)),
    )
    # distribute the shared-expert tokens: flex-shared cores get F tokens in
    # their flex segment, every core gets up to B in its shared segment
    sh_pos = 0
    in_maps = []
    core_meta = []
    for c in range(N_CORES):
        idx = idx_e[c]
        exp_n = min(len(idx), A)
        over_n = len(idx) - exp_n  # >0 only when use_flex and this expert overflows
        ew = (
            _wT_layout(w1[c].astype(bfloat16)),
            _wT_layout(w3[c].astype(bfloat16)),
            _w2T_layout(w2[c].astype(bfloat16)),
        )
        tok = np.zeros((N, DIM), dtype=bfloat16)
        tok[:exp_n] = x_bf[idx[:exp_n]]
        m = {
            "xT": None,
            "w1T": ew[0], "w3T": ew[1], "w2T": ew[2],
            "s1T": sh_w[0], "s3T": sh_w[1], "s2T": sh_w[2],
        }
        flex_sh_idx = None
        if F:
            if over_n:
                tok[A:A + over_n] = x_bf[idx[exp_n:]]
                m["f1T"], m["f3T"], m["f2T"] = ew
            else:
                fn = min(F, T - sh_pos)
                flex_sh_idx = np.arange(sh_pos, sh_pos + fn)
                tok[A:A + fn] = x_bf[flex_sh_idx]
                sh_pos += fn
                m["f1T"], m["f3T"], m["f2T"] = sh_w
        bn = min(BSEG, T - sh_pos)
        sh_idx = np.arange(sh_pos, sh_pos + bn)
        tok[A + F:A + F + bn] = x_bf[sh_idx]
        sh_pos += bn
        m["xT"] = _xT_layout(tok, N)
        in_maps.append(m)
        core_meta.append((exp_n, over_n, flex_sh_idx, sh_idx))
    assert sh_pos == T, f"shared token distribution bug: {sh_pos} != {T}"

    _CACHE["last_in_maps"] = in_maps
    res = run_bass_kernel_spmd(nc, in_maps, list(range(N_CORES)))

    y = np.zeros((T, DIM), dtype=np.float32)
    for c in range(N_CORES):
        yc_dm = np.asarray(res.results[c]["y"], dtype=np.float32)  # [128, DC, N]
        yc = yc_dm.transpose(1, 0, 2).reshape(DIM, N).T  # [N, DIM]
        idx = idx_e[c]
        exp_n, over_n, flex_sh_idx, sh_idx = core_meta[c]
        y[idx[:exp_n]] += cw_e[c][:exp_n, None] * yc[:exp_n]
        if over_n:
            y[idx[exp_n:]] += cw_e[c][exp_n:, None] * yc[A:A + over_n]
        elif flex_sh_idx is not None and len(flex_sh_idx):
            y[flex_sh_idx] += yc[A:A + len(flex_sh_idx)]
        y[sh_idx] += yc[A + F:A + F + len(sh_idx)]
    return y.reshape(B, S, D)


# revision 52
# speedup vs baseline: 1.0133x; 1.0079x over previous
"""Bass/Trainium2 kernel for nn_MOEFeedForward (8-expert top-2 MoE + shared expert).

Sharding: expert-parallel with host-side dispatch. The host computes the gate
(softmax + top-2) and routes tokens: core c receives expert c's tokens (padded
to capacity A = ceil8(max expert load)) plus a 1/8 token-slice of the
shared-expert work (B = 256 tokens). Every core runs A+B token-FFN columns of
identical shape (hid=2048, dim=768) — balanced, no 8x dense overcompute. The
host applies the gate weights and scatter-adds per-core outputs into the full
result.

Device kernel: all operands pre-transposed/laid out on the host so the device
does only contiguous DMAs and back-to-back bf16 matmuls at 1 col/cycle.
Column pieces of <=512 run mm1/mm3 (hid-chunked, PSUM-accumulated over the 6
dim-chunks), silu*mul drains to bf16 hT, then mm2 in d-major form
(y[d, t], 6 dim psums contracting 16 hid chunks). Dummy PE warmup matmuls
ramp the tensor-engine clock while the first DMAs land. Cost-model makespan
~105.8us/core vs ~99.8us pure-matmul floor at 2.4 GHz.

Self-contained: hardcodes shapes from the problem spec.
"""
import os
import sys

sys.path.insert(0, "/opt/trn_rl_repo")

from contextlib import ExitStack

import numpy as np
from ml_dtypes import bfloat16

import concourse.bass as bass
import concourse.tile as tile
from concourse import mybir
from concourse.bass_utils import run_bass_kernel_spmd
from concourse.vector_clock import ScopedClock

DIM = 768
HID = 2048
E = 8
T = 2048
N_CORES = 8
B_SH = T // N_CORES  # shared-expert tokens per core (256)
DC = DIM // 128      # 6 d-chunks
HC = HID // 128      # 16 hid-chunks

F32 = mybir.dt.float32
BF16 = mybir.dt.bfloat16

AF = mybir.ActivationFunctionType
OP = mybir.AluOpType


# ---------------------------------------------------------------------------
# Walrus in this container rejects CTRL instructions (NoOp/Drain) carrying
# more than one sem wait. TileContext's tail drain carries one wait per
# outstanding semaphore. Replace it with a chain of SP nops (one wait each)
# followed by a bare drain.
def _patched_drain_and_barrier(self, tick_clock, wait_clock):
    import bass_rust

    nop_inst = self.nc.sync.nop(nofuse=True, hint="pre_drain_wait_funnel")
    wait_clock.add_sem_waits(
        nop_inst.ins, ScopedClock({None: tick_clock.global_clock})
    )
    si = nop_inst.ins.sync_info
    waits = list(si.on_wait) if si else []
    if len(waits) > 1:
        nop_inst.ins.sync_info.on_wait = waits[:1]
        for w in waits[1:]:
            extra = self.nc.sync.nop(nofuse=True, hint="pre_drain_wait_funnel")
            extra.ins.sync_info = bass_rust.SyncInfo(on_wait=[w], on_update=[])
    self.nc.sync.drain()

    self.nc.all_engine_barrier()
    assert self.sems is not None
    popped = self.nc._tile_sem_poison_stack.pop()
    assert popped is self._sem_poison
    self.nc.clear_and_free_semaphores(list(self.sems.allocated().values()))
    self.nc.all_engine_barrier()


tile.TileContext._drain_and_barrier = _patched_drain_and_barrier


def _split_multi_waits(nc, max_waits=1):
    """This walrus build allows at most one sem wait per instruction. Hoist
    extra waits onto same-engine nops inserted immediately before."""
    import bass_rust

    n_split = 0
    for f in nc.m.functions:
        for bb in f.blocks:
            il = bb.instructions
            i = 0
            while i < len(il):
                inst = il[i]
                si = inst.sync_info
                if si is None or len(si.on_wait) <= max_waits:
                    i += 1
                    continue
                waits = list(si.on_wait)
                si.on_wait = waits[:max_waits]
                for k, w in enumerate(waits[max_waits:]):
                    nop = mybir.InstNoOp(
                        name=f"{inst.name}-wsplit{k}", ins=[], outs=[]
                    )
                    nop.engine = inst.engine
                    nop.sync_info = bass_rust.SyncInfo(on_wait=[w], on_update=[])
                    il.insert(i, nop)
                    i += 1
                n_split += 1
                i += 1
    return n_split
# ---------------------------------------------------------------------------


def _build_kernel(A, reps=1, F=0, B=B_SH):
    """A: expert-token capacity. Columns [0, A) use the expert weight set;
    with F>0, columns [A, A+F) use a per-core 'flex' weight set (host fills
    with either this core's expert weights or the shared weights); columns
    [A+F, A+F+B) use the shared weight set.
    Output y is d-major: y_d[p, dc, t] = y[t, dc*128+p].
    reps>1 repeats the whole compute (for benchmarking)."""
    N = A + F + B
    nc = bass.Bass()
    xT_d = nc.dram_tensor("xT", [128, DC, N], BF16, kind="ExternalInput")
    w1_d = nc.dram_tensor("w1T", [128, HC, DC, 128], BF16, kind="ExternalInput")
    w3_d = nc.dram_tensor("w3T", [128, HC, DC, 128], BF16, kind="ExternalInput")
    w2_d = nc.dram_tensor("w2T", [128, HC, DC, 128], BF16, kind="ExternalInput")
    s1_d = nc.dram_tensor("s1T", [128, HC, DC, 128], BF16, kind="ExternalInput")
    s3_d = nc.dram_tensor("s3T", [128, HC, DC, 128], BF16, kind="ExternalInput")
    s2_d = nc.dram_tensor("s2T", [128, HC, DC, 128], BF16, kind="ExternalInput")
    if F:
        f1_d = nc.dram_tensor("f1T", [128, HC, DC, 128], BF16, kind="ExternalInput")
        f3_d = nc.dram_tensor("f3T", [128, HC, DC, 128], BF16, kind="ExternalInput")
        f2_d = nc.dram_tensor("f2T", [128, HC, DC, 128], BF16, kind="ExternalInput")
    y_d = nc.dram_tensor("y", [128, DC, N], F32, kind="ExternalOutput")

    # column pieces: (start, len, weight-set)
    first = int(os.environ.get("K_P0", "512"))
    exp_pieces = []
    c0 = 0
    while c0 < A:
        ln = min(first if c0 == 0 else 512, A - c0)
        exp_pieces.append((c0, ln, 0))
        c0 += ln
    # flex is processed LAST: its weights reuse w1/w3/w2's SBUF (same pool
    # tags), so their DMA can only land after the expert pieces finish.
    pieces = exp_pieces + [(A + F, B, 1)]
    if F:
        pieces = pieces + [(A, F, 2)]

    _g = lambda k, d: int(os.environ.get(k, str(d)))
    with tile.TileContext(nc) as tc, ExitStack() as ctx:
        persist = ctx.enter_context(tc.tile_pool(name="persist", bufs=1))
        silu_p = ctx.enter_context(tc.tile_pool(name="silu", bufs=_g("K_SILU", 3)))
        yo_p = ctx.enter_context(tc.tile_pool(name="yo", bufs=_g("K_YO", 3)))
        h5 = os.environ.get("K_H5", "1") == "1"
        h_ps = ctx.enter_context(
            tc.tile_pool(name="h_ps", bufs=_g("K_HPS", 5 if h5 else 4), space="PSUM"))
        y_ps = ctx.enter_context(
            tc.tile_pool(name="y_ps", bufs=_g("K_YPS", 3), space="PSUM"))
        if h5:
            wup_ps = y_ps  # warmup psum borrows the y ring; its bank goes to h_ps
        else:
            wup_ps = ctx.enter_context(
                tc.tile_pool(name="wup_ps", bufs=1, space="PSUM"))

        xT = persist.tile([128, DC, N], BF16, tag="xT")
        w1T = persist.tile([128, HC, DC, 128], BF16, tag="w1T")
        w3T = persist.tile([128, HC, DC, 128], BF16, tag="w3T")
        s1T = persist.tile([128, HC, DC, 128], BF16, tag="s1T")
        s3T = persist.tile([128, HC, DC, 128], BF16, tag="s3T")
        w2T = persist.tile([128, HC, DC, 128], BF16, tag="w2T")
        s2T = persist.tile([128, HC, DC, 128], BF16, tag="s2T")
        # hT holds one piece's activations [hid, piece_cols]; small pieces
        # (<=64 cols) get their own tile so piece transitions don't WAR-chain
        hT = persist.tile([128, HC, 512], BF16, tag="hT")
        hT2 = None
        if os.environ.get("K_HT2", "0") == "1":
            small = [p for p in pieces if p[1] <= 64]
            if small:
                hT2 = persist.tile([128, HC, max(p[1] for p in small)],
                                   BF16, tag="hT2")

        # --- PE warmup: dummy matmuls with no data deps keep the tensor
        # engine busy (and its clock ramping) while the first DMAs land.
        n_wup = _g("K_WUP", 6)
        if n_wup:
            wup = persist.tile([128, 512], BF16, tag="wup")
            nc.vector.memset(wup[:], 0)
            wup_p = wup_ps.tile([128, 512], F32, tag="wup" if not h5 else "y")
            for _ in range(n_wup):
                nc.tensor.matmul(wup_p[:], wup[:, 0:128], wup[:], start=True, stop=True)

        f_d = (f1_d, f3_d, f2_d) if F else None
        for _rep in range(reps):
            _kernel_body(nc, pieces, N, xT_d, w1_d, w3_d, w2_d, s1_d, s3_d,
                         s2_d, y_d, xT, w1T, w3T, s1T, s3T, w2T, s2T, hT,
                         silu_p, yo_p, h_ps, y_ps, persist, f_d, wup_ps, hT2)

    _split_multi_waits(nc)
    try:
        _CACHE["makespan_ns"] = max(e[2] for e in tc._perfetto_entries)
    except Exception:
        _CACHE["makespan_ns"] = None
    return nc


def _kernel_body(nc, pieces, N, xT_d, w1_d, w3_d, w2_d, s1_d, s3_d, s2_d,
                 y_d, xT, w1T, w3T, s1T, s3T, w2T, s2T, hT,
                 silu_p, yo_p, h_ps, y_ps, persist=None, f_d=None, wup_ps=None,
                 hT2=None):
    if True:
        # --- DMA schedule: first-piece x interleaved with first w1/w3
        # hid-chunks (PE consumes chunk-by-chunk), then the rest.
        c00, c0len = pieces[0][0], pieces[0][1]
        pro = os.environ.get("K_PRO", "0")
        if pro == "2":
            # all of x first (PE is in warmup anyway), then w1/w3 pairs —
            # the first h series then never waits on an x chunk
            for dc in range(DC):
                nc.sync.dma_start(
                    xT[:, dc, c00:c00 + c0len], xT_d[:, dc, c00:c00 + c0len]
                )
            for hc in range(2):
                nc.sync.dma_start(w1T[:, hc], w1_d[:, hc])
                nc.sync.dma_start(w3T[:, hc], w3_d[:, hc])
        elif pro == "1":
            # w1 hc0 first so the first p1 series starts as x chunks stream in
            nc.sync.dma_start(w1T[:, 0], w1_d[:, 0])
            for dc in range(DC):
                nc.sync.dma_start(
                    xT[:, dc, c00:c00 + c0len], xT_d[:, dc, c00:c00 + c0len]
                )
            nc.sync.dma_start(w3T[:, 0], w3_d[:, 0])
            nc.sync.dma_start(w1T[:, 1], w1_d[:, 1])
            nc.sync.dma_start(w3T[:, 1], w3_d[:, 1])
        else:
            for dc in range(DC):
                nc.sync.dma_start(
                    xT[:, dc, c00:c00 + c0len], xT_d[:, dc, c00:c00 + c0len]
                )
                if dc < 2:
                    nc.sync.dma_start(w1T[:, dc], w1_d[:, dc])
                    nc.sync.dma_start(w3T[:, dc], w3_d[:, dc])
        for hc in range(2, HC):
            nc.sync.dma_start(w1T[:, hc], w1_d[:, hc])
            nc.sync.dma_start(w3T[:, hc], w3_d[:, hc])
            if hc == 7 and c0len < N:
                nc.sync.dma_start(xT[:, :, c0len:N], xT_d[:, :, c0len:N])
        nc.sync.dma_start(w2T[:], w2_d[:])
        nc.sync.dma_start(s1T[:], s1_d[:])
        nc.sync.dma_start(s3T[:], s3_d[:])
        nc.sync.dma_start(s2T[:], s2_d[:])
        if f_d is not None:
            # flex weights reuse the expert-weight SBUF slots (same tags,
            # bufs=1 ring): the DMAs wait for the expert pieces' last reads.
            f1T = persist.tile([128, HC, DC, 128], BF16, tag="w1T")
            f3T = persist.tile([128, HC, DC, 128], BF16, tag="w3T")
            f2T = persist.tile([128, HC, DC, 128], BF16, tag="w2T")
            for hc in range(HC):
                nc.sync.dma_start(f1T[:, hc], f_d[0][:, hc])
                nc.sync.dma_start(f3T[:, hc], f_d[1][:, hc])
            nc.sync.dma_start(f2T[:], f_d[2][:])

        n_borrow = int(os.environ.get("K_BORROW", "0"))
        psum_dma = os.environ.get("K_PSDMA", "0") == "1"
        for pi, (c0, clen, ws) in enumerate(pieces):
            if ws == 0:
                a1T, a3T, a2T = (w1T, w3T, w2T)
            elif ws == 1:
                a1T, a3T, a2T = (s1T, s3T, s2T)
            else:
                a1T, a3T, a2T = (f1T, f3T, f2T)
            ht = hT2 if (hT2 is not None and clen <= 64) else hT
            csl = slice(c0, c0 + clen)
            # h = silu(w1 x) * (w3 x), written to hT[:, :, 0:clen]
            for hc in range(HC):
                # y_ps banks are idle until the first mm2 — borrow them for
                # the first few h chunks so the startup WAR chain never waits
                hp = y_ps if (pi == 0 and hc < n_borrow) else h_ps
                p1 = hp.tile([128, 512], F32, tag="hps" if hp is h_ps else "y")
                for dc in range(DC):
                    nc.tensor.matmul(
                        p1[:, 0:clen], a1T[:, hc, dc], xT[:, dc, csl],
                        start=(dc == 0), stop=(dc == DC - 1),
                    )
                p3 = hp.tile([128, 512], F32, tag="hps" if hp is h_ps else "y")
                for dc in range(DC):
                    nc.tensor.matmul(
                        p3[:, 0:clen], a3T[:, hc, dc], xT[:, dc, csl],
                        start=(dc == 0), stop=(dc == DC - 1),
                    )
                sl = silu_p.tile([128, 512], BF16, tag="silu")
                nc.scalar.activation(sl[:, 0:clen], p1[:, 0:clen], AF.Silu)
                nc.vector.tensor_tensor(
                    ht[:, hc, 0:clen], sl[:, 0:clen], p3[:, 0:clen], op=OP.mult
                )

            # y[d, t] = w2.T h for this piece (d-major output)
            for dc in range(DC):
                if (wup_ps is not y_ps
                        and os.environ.get("K_WUPY", "1") == "1"
                        and dc % 4 == 3):
                    py = wup_ps.tile([128, 512], F32, tag="wup")
                else:
                    py = y_ps.tile([128, 512], F32, tag="y")
                for hc in range(HC):
                    nc.tensor.matmul(
                        py[:, 0:clen], a2T[:, hc, dc], ht[:, hc, 0:clen],
                        start=(hc == 0), stop=(hc == HC - 1),
                    )
                if psum_dma:
                    nc.sync.dma_start(y_d[:, dc, csl], py[:, 0:clen])
                else:
                    yo = yo_p.tile([128, 512], F32, tag="yo")
                    nc.vector.tensor_copy(yo[:, 0:clen], py[:, 0:clen])
                    nc.sync.dma_start(y_d[:, dc, csl], yo[:, 0:clen])


_CACHE = {}


def _wT_layout(w):
    """[HID, DIM] (bf16) -> DRAM layout [128, HC, DC, 128] where
    [p, hc, dc, i] = w[hc*128 + i, dc*128 + p]."""
    return np.ascontiguousarray(
        w.reshape(HC, 128, DC, 128).transpose(3, 0, 2, 1)
    )


def _w2T_layout(w):
    """[DIM, HID] (bf16) -> DRAM layout [128, HC, DC, 128] where
    [p, hc, dc, i] = w[dc*128 + i, hc*128 + p]."""
    return np.ascontiguousarray(
        w.T.reshape(HC, 128, DC, 128).transpose(1, 0, 2, 3)
    )


def _xT_layout(tok, N):
    """[N, DIM] (bf16) -> DRAM layout [128, DC, N]."""
    return np.ascontiguousarray(tok.T.reshape(DC, 128, N).transpose(1, 0, 2))


def kernel(x, gate_w, w1, w2, w3, ws1, ws2, ws3):
    x = np.asarray(x, dtype=np.float32)
    gate_w = np.asarray(gate_w, dtype=np.float32)
    w1 = np.asarray(w1, dtype=np.float32)
    w2 = np.asarray(w2, dtype=np.float32)
    w3 = np.asarray(w3, dtype=np.float32)
    ws1 = np.asarray(ws1, dtype=np.float32)
    ws2 = np.asarray(ws2, dtype=np.float32)
    ws3 = np.asarray(ws3, dtype=np.float32)

    B, S, D = x.shape
    x2 = np.ascontiguousarray(x.reshape(-1, D))
    Tn = x2.shape[0]
    assert Tn == T and D == DIM

    # --- gate: softmax + top-2 + weight normalization (host)
    logits = x2 @ gate_w.T
    m = logits.max(-1, keepdims=True)
    sm = np.exp(logits - m)
    sm /= sm.sum(-1, keepdims=True)
    ti = np.argsort(-sm, axis=-1)[:, :2]
    tw = np.take_along_axis(sm, ti, axis=-1)
    tw = tw / (tw.sum(-1, keepdims=True) + 1e-20)

    idx_e, cw_e = [], []
    for e in range(E):
        sel = (ti[:, 0] == e) | (ti[:, 1] == e)
        idx = np.nonzero(sel)[0]
        w_tok = np.where(ti[idx, 0] == e, tw[idx, 0], 0.0) + np.where(
            ti[idx, 1] == e, tw[idx, 1], 0.0
        )
        idx_e.append(idx)
        cw_e.append(w_tok.astype(np.float32))

    maxL = max(len(i) for i in idx_e)
    # A true flex segment (filling pad columns with shared work) costs a full
    # extra weight-set DMA per core and models worse (113.7us vs 105.8us);
    # kept only behind K_FLEX=1 for experiments.
    use_flex = os.environ.get("K_FLEX", "0") == "1" and maxL > 512
    if use_flex:
        A = 512
        F = -(-(maxL - A) // 8) * 8
        n_over = sum(1 for i in idx_e if len(i) > A)
        BSEG = -(-(T - (N_CORES - n_over) * F) // N_CORES // 8) * 8
    else:
        A = max(128, -(-maxL // 8) * 8)
        F = 0
        BSEG = B_SH
    N = A + F + BSEG

    key = ("nc", A, F, BSEG)
    if key not in _CACHE:
        _CACHE[key] = _build_kernel(A, F=F, B=BSEG)
    nc = _CACHE[key]
    _CACHE["nc"] = nc

    x_bf = x2.astype(bfloat16)
    sh_w = (
        _wT_layout(ws1.astype(bfloat16)),
        _wT_layout(ws3.astype(bfloat16)),
        _w2T_layout(ws2.astype(bfloat16)),
    )
    # distribute the shared-expert tokens: flex-shared cores get F tokens in
    # their flex segment, every core gets up to B in its shared segment
    sh_pos = 0
    in_maps = []
    core_meta = []
    for c in range(N_CORES):
        idx = idx_e[c]
        exp_n = min(len(idx), A)
        over_n = len(idx) - exp_n  # >0 only when use_flex and this expert overflows
        ew = (
            _wT_layout(w1[c].astype(bfloat16)),
            _wT_layout(w3[c].astype(bfloat16)),
            _w2T_layout(w2[c].astype(bfloat16)),
        )
        tok = np.zeros((N, DIM), dtype=bfloat16)
        tok[:exp_n] = x_bf[idx[:exp_n]]
        m = {
            "xT": None,
            "w1T": ew[0], "w3T": ew[1], "w2T": ew[2],
            "s1T": sh_w[0], "s3T": sh_w[1], "s2T": sh_w[2],
        }
        flex_sh_idx = None
        if F:
            if over_n:
                tok[A:A + over_n] = x_bf[idx[exp_n:]]
                m["f1T"], m["f3T"], m["f2T"] = ew
            else:
                fn = min(F, T - sh_pos)
                flex_sh_idx = np.arange(sh_pos, sh_pos + fn)
                tok[A:A + fn] = x_bf[flex_sh_idx]
                sh_pos += fn
                m["f1T"], m["f3T"], m["f2T"] = sh_w
        bn = min(BSEG, T - sh_pos)
        sh_idx = np.arange(sh_pos, sh_pos + bn)
        tok[A + F:A + F + bn] = x_bf[sh_idx]
        sh_pos += bn
        m["xT"] = _xT_layout(tok, N)
        in_maps.append(m)
        core_meta.append((exp_n, over_n, flex_sh_idx, sh_idx))
    assert sh_pos == T, f"shared token distribution bug: {sh_pos} != {T}"

    _CACHE["last_in_maps"] = in_maps
    res = run_bass_kernel_spmd(nc, in_maps, list(range(N_CORES)))

    y = np.zeros((T, DIM), dtype=np.float32)
    for c in range(N_CORES):
        yc_dm = np.asarray(res.results[c]["y"], dtype=np.float32)  # [128, DC, N]
        yc = yc_dm.transpose(1, 0, 2).reshape(DIM, N).T  # [N, DIM]
        idx = idx_e[c]
        exp_n, over_n, flex_sh_idx, sh_idx = core_meta[c]
        y[idx[:exp_n]] += cw_e[c][:exp_n, None] * yc[:exp_n]
        if over_n:
            y[idx[exp_n:]] += cw_e[c][exp_n:, None] * yc[A:A + over_n]
        elif flex_sh_idx is not None and len(flex_sh_idx):
            y[flex_sh_idx] += yc[A:A + len(flex_sh_idx)]
        y[sh_idx] += yc[A + F:A + F + len(sh_idx)]
    return y.reshape(B, S, D)


# revision 54
# speedup vs baseline: 1.0148x; 1.0014x over previous
"""Bass/Trainium2 kernel for nn_MOEFeedForward (8-expert top-2 MoE + shared expert).

Sharding: expert-parallel with host-side dispatch. The host computes the gate
(softmax + top-2) and routes tokens: core c receives expert c's tokens (padded
to capacity A = ceil8(max expert load)) plus a 1/8 token-slice of the
shared-expert work (B = 256 tokens). Every core runs A+B token-FFN columns of
identical shape (hid=2048, dim=768) — balanced, no 8x dense overcompute. The
host applies the gate weights and scatter-adds per-core outputs into the full
result.

Device kernel: all operands pre-transposed/laid out on the host so the device
does only contiguous DMAs and back-to-back bf16 matmuls at 1 col/cycle.
Column pieces of <=512 run mm1/mm3 (hid-chunked, PSUM-accumulated over the 6
dim-chunks), silu*mul drains to bf16 hT, then mm2 in d-major form
(y[d, t], 6 dim psums contracting 16 hid chunks). Dummy PE warmup matmuls
ramp the tensor-engine clock while the first DMAs land. Cost-model makespan
~105.8us/core vs ~99.8us pure-matmul floor at 2.4 GHz.

Self-contained: hardcodes shapes from the problem spec.
"""
import os
import sys

sys.path.insert(0, "/opt/trn_rl_repo")

from contextlib import ExitStack

import numpy as np
from ml_dtypes import bfloat16

import concourse.bass as bass
import concourse.tile as tile
from concourse import mybir
from concourse.bass_utils import run_bass_kernel_spmd
from concourse.vector_clock import ScopedClock

DIM = 768
HID = 2048
E = 8
T = 2048
N_CORES = 8
B_SH = T // N_CORES  # shared-expert tokens per core (256)
DC = DIM // 128      # 6 d-chunks
HC = HID // 128      # 16 hid-chunks

F32 = mybir.dt.float32
BF16 = mybir.dt.bfloat16

AF = mybir.ActivationFunctionType
OP = mybir.AluOpType


# ---------------------------------------------------------------------------
# Walrus in this container rejects CTRL instructions (NoOp/Drain) carrying
# more than one sem wait. TileContext's tail drain carries one wait per
# outstanding semaphore. Replace it with a chain of SP nops (one wait each)
# followed by a bare drain.
def _patched_drain_and_barrier(self, tick_clock, wait_clock):
    import bass_rust

    nop_inst = self.nc.sync.nop(nofuse=True, hint="pre_drain_wait_funnel")
    wait_clock.add_sem_waits(
        nop_inst.ins, ScopedClock({None: tick_clock.global_clock})
    )
    si = nop_inst.ins.sync_info
    waits = list(si.on_wait) if si else []
    if len(waits) > 1:
        nop_inst.ins.sync_info.on_wait = waits[:1]
        for w in waits[1:]:
            extra = self.nc.sync.nop(nofuse=True, hint="pre_drain_wait_funnel")
            extra.ins.sync_info = bass_rust.SyncInfo(on_wait=[w], on_update=[])
    self.nc.sync.drain()

    self.nc.all_engine_barrier()
    assert self.sems is not None
    popped = self.nc._tile_sem_poison_stack.pop()
    assert popped is self._sem_poison
    self.nc.clear_and_free_semaphores(list(self.sems.allocated().values()))
    self.nc.all_engine_barrier()


tile.TileContext._drain_and_barrier = _patched_drain_and_barrier


def _split_multi_waits(nc, max_waits=1):
    """This walrus build allows at most one sem wait per instruction. Hoist
    extra waits onto same-engine nops inserted immediately before."""
    import bass_rust

    n_split = 0
    for f in nc.m.functions:
        for bb in f.blocks:
            il = bb.instructions
            i = 0
            while i < len(il):
                inst = il[i]
                si = inst.sync_info
                if si is None or len(si.on_wait) <= max_waits:
                    i += 1
                    continue
                waits = list(si.on_wait)
                si.on_wait = waits[:max_waits]
                for k, w in enumerate(waits[max_waits:]):
                    nop = mybir.InstNoOp(
                        name=f"{inst.name}-wsplit{k}", ins=[], outs=[]
                    )
                    nop.engine = inst.engine
                    nop.sync_info = bass_rust.SyncInfo(on_wait=[w], on_update=[])
                    il.insert(i, nop)
                    i += 1
                n_split += 1
                i += 1
    return n_split
# ---------------------------------------------------------------------------


def _build_kernel(A, reps=1, F=0, B=B_SH):
    """A: expert-token capacity. Columns [0, A) use the expert weight set;
    with F>0, columns [A, A+F) use a per-core 'flex' weight set (host fills
    with either this core's expert weights or the shared weights); columns
    [A+F, A+F+B) use the shared weight set.
    Output y is d-major: y_d[p, dc, t] = y[t, dc*128+p].
    reps>1 repeats the whole compute (for benchmarking)."""
    N = A + F + B
    nc = bass.Bass()
    xT_d = nc.dram_tensor("xT", [128, DC, N], BF16, kind="ExternalInput")
    w1_d = nc.dram_tensor("w1T", [128, HC, DC, 128], BF16, kind="ExternalInput")
    w3_d = nc.dram_tensor("w3T", [128, HC, DC, 128], BF16, kind="ExternalInput")
    w2_d = nc.dram_tensor("w2T", [128, HC, DC, 128], BF16, kind="ExternalInput")
    s1_d = nc.dram_tensor("s1T", [128, HC, DC, 128], BF16, kind="ExternalInput")
    s3_d = nc.dram_tensor("s3T", [128, HC, DC, 128], BF16, kind="ExternalInput")
    s2_d = nc.dram_tensor("s2T", [128, HC, DC, 128], BF16, kind="ExternalInput")
    if F:
        f1_d = nc.dram_tensor("f1T", [128, HC, DC, 128], BF16, kind="ExternalInput")
        f3_d = nc.dram_tensor("f3T", [128, HC, DC, 128], BF16, kind="ExternalInput")
        f2_d = nc.dram_tensor("f2T", [128, HC, DC, 128], BF16, kind="ExternalInput")
    y_d = nc.dram_tensor("y", [128, DC, N], F32, kind="ExternalOutput")

    # column pieces: (start, len, weight-set)
    first = int(os.environ.get("K_P0", "512"))
    exp_pieces = []
    c0 = 0
    while c0 < A:
        ln = min(first if c0 == 0 else 512, A - c0)
        exp_pieces.append((c0, ln, 0))
        c0 += ln
    # flex is processed LAST: its weights reuse w1/w3/w2's SBUF (same pool
    # tags), so their DMA can only land after the expert pieces finish.
    pieces = exp_pieces + [(A + F, B, 1)]
    if F:
        pieces = pieces + [(A, F, 2)]

    _g = lambda k, d: int(os.environ.get(k, str(d)))
    with tile.TileContext(nc) as tc, ExitStack() as ctx:
        persist = ctx.enter_context(tc.tile_pool(name="persist", bufs=1))
        if os.environ.get("K_MERGE", "0") == "1":
            yo_p = ctx.enter_context(tc.tile_pool(name="yo", bufs=_g("K_YO", 3)))
            silu_p = yo_p
        else:
            silu_p = ctx.enter_context(
                tc.tile_pool(name="silu", bufs=_g("K_SILU", 3)))
            yo_p = ctx.enter_context(tc.tile_pool(name="yo", bufs=_g("K_YO", 3)))
        h5 = os.environ.get("K_H5", "1") == "1"
        h_ps = ctx.enter_context(
            tc.tile_pool(name="h_ps", bufs=_g("K_HPS", 5 if h5 else 4), space="PSUM"))
        y_ps = ctx.enter_context(
            tc.tile_pool(name="y_ps", bufs=_g("K_YPS", 3), space="PSUM"))
        if h5:
            wup_ps = y_ps  # warmup psum borrows the y ring; its bank goes to h_ps
        else:
            wup_ps = ctx.enter_context(
                tc.tile_pool(name="wup_ps", bufs=1, space="PSUM"))

        xT = persist.tile([128, DC, N], BF16, tag="xT")
        w1T = persist.tile([128, HC, DC, 128], BF16, tag="w1T")
        w3T = persist.tile([128, HC, DC, 128], BF16, tag="w3T")
        s1T = persist.tile([128, HC, DC, 128], BF16, tag="s1T")
        s3T = persist.tile([128, HC, DC, 128], BF16, tag="s3T")
        w2T = persist.tile([128, HC, DC, 128], BF16, tag="w2T")
        s2T = persist.tile([128, HC, DC, 128], BF16, tag="s2T")
        # hT holds one piece's activations [hid, piece_cols]; small pieces
        # (<=64 cols) get their own tile so piece transitions don't WAR-chain
        hT = persist.tile([128, HC, 512], BF16, tag="hT")
        hT2 = None
        if os.environ.get("K_HT2", "0") == "1":
            small = [p for p in pieces if p[1] <= 64]
            if small:
                hT2 = persist.tile([128, HC, max(p[1] for p in small)],
                                   BF16, tag="hT2")

        # --- PE warmup: dummy matmuls with no data deps keep the tensor
        # engine busy (and its clock ramping) while the first DMAs land.
        n_wup = _g("K_WUP", 6)
        if n_wup:
            wup = persist.tile([128, 512], BF16, tag="wup")
            nc.vector.memset(wup[:], 0)
            wup_p = wup_ps.tile([128, 512], F32, tag="wup" if not h5 else "y")
            for _ in range(n_wup):
                nc.tensor.matmul(wup_p[:], wup[:, 0:128], wup[:], start=True, stop=True)

        f_d = (f1_d, f3_d, f2_d) if F else None
        for _rep in range(reps):
            _kernel_body(nc, pieces, N, xT_d, w1_d, w3_d, w2_d, s1_d, s3_d,
                         s2_d, y_d, xT, w1T, w3T, s1T, s3T, w2T, s2T, hT,
                         silu_p, yo_p, h_ps, y_ps, persist, f_d, wup_ps, hT2)

    _split_multi_waits(nc)
    try:
        _CACHE["makespan_ns"] = max(e[2] for e in tc._perfetto_entries)
    except Exception:
        _CACHE["makespan_ns"] = None
    return nc


def _kernel_body(nc, pieces, N, xT_d, w1_d, w3_d, w2_d, s1_d, s3_d, s2_d,
                 y_d, xT, w1T, w3T, s1T, s3T, w2T, s2T, hT,
                 silu_p, yo_p, h_ps, y_ps, persist=None, f_d=None, wup_ps=None,
                 hT2=None):
    if True:
        # --- DMA schedule: first-piece x interleaved with first w1/w3
        # hid-chunks (PE consumes chunk-by-chunk), then the rest.
        c00, c0len = pieces[0][0], pieces[0][1]
        pro = os.environ.get("K_PRO", "0")
        if pro == "2":
            # all of x first (PE is in warmup anyway), then w1/w3 pairs —
            # the first h series then never waits on an x chunk
            for dc in range(DC):
                nc.sync.dma_start(
                    xT[:, dc, c00:c00 + c0len], xT_d[:, dc, c00:c00 + c0len]
                )
            for hc in range(2):
                nc.sync.dma_start(w1T[:, hc], w1_d[:, hc])
                nc.sync.dma_start(w3T[:, hc], w3_d[:, hc])
        elif pro == "1":
            # w1 hc0 first so the first p1 series starts as x chunks stream in
            nc.sync.dma_start(w1T[:, 0], w1_d[:, 0])
            for dc in range(DC):
                nc.sync.dma_start(
                    xT[:, dc, c00:c00 + c0len], xT_d[:, dc, c00:c00 + c0len]
                )
            nc.sync.dma_start(w3T[:, 0], w3_d[:, 0])
            nc.sync.dma_start(w1T[:, 1], w1_d[:, 1])
            nc.sync.dma_start(w3T[:, 1], w3_d[:, 1])
        else:
            for dc in range(DC):
                nc.sync.dma_start(
                    xT[:, dc, c00:c00 + c0len], xT_d[:, dc, c00:c00 + c0len]
                )
                if dc < 2:
                    nc.sync.dma_start(w1T[:, dc], w1_d[:, dc])
                    nc.sync.dma_start(w3T[:, dc], w3_d[:, dc])
        for hc in range(2, HC):
            nc.sync.dma_start(w1T[:, hc], w1_d[:, hc])
            nc.sync.dma_start(w3T[:, hc], w3_d[:, hc])
            if hc == 7 and c0len < N:
                nc.sync.dma_start(xT[:, :, c0len:N], xT_d[:, :, c0len:N])
        nc.sync.dma_start(w2T[:], w2_d[:])
        nc.sync.dma_start(s1T[:], s1_d[:])
        nc.sync.dma_start(s3T[:], s3_d[:])
        nc.sync.dma_start(s2T[:], s2_d[:])
        if f_d is not None:
            # flex weights reuse the expert-weight SBUF slots (same tags,
            # bufs=1 ring): the DMAs wait for the expert pieces' last reads.
            f1T = persist.tile([128, HC, DC, 128], BF16, tag="w1T")
            f3T = persist.tile([128, HC, DC, 128], BF16, tag="w3T")
            f2T = persist.tile([128, HC, DC, 128], BF16, tag="w2T")
            for hc in range(HC):
                nc.sync.dma_start(f1T[:, hc], f_d[0][:, hc])
                nc.sync.dma_start(f3T[:, hc], f_d[1][:, hc])
            nc.sync.dma_start(f2T[:], f_d[2][:])

        n_borrow = int(os.environ.get("K_BORROW", "0"))
        psum_dma = os.environ.get("K_PSDMA", "0") == "1"
        for pi, (c0, clen, ws) in enumerate(pieces):
            if ws == 0:
                a1T, a3T, a2T = (w1T, w3T, w2T)
            elif ws == 1:
                a1T, a3T, a2T = (s1T, s3T, s2T)
            else:
                a1T, a3T, a2T = (f1T, f3T, f2T)
            ht = hT2 if (hT2 is not None and clen <= 64) else hT
            csl = slice(c0, c0 + clen)
            # h = silu(w1 x) * (w3 x), written to hT[:, :, 0:clen]
            for hc in range(HC):
                # y_ps banks are idle until the first mm2 — borrow them for
                # the first few h chunks so the startup WAR chain never waits
                hp = y_ps if (pi == 0 and hc < n_borrow) else h_ps
                p1 = hp.tile([128, 512], F32, tag="hps" if hp is h_ps else "y")
                for dc in range(DC):
                    nc.tensor.matmul(
                        p1[:, 0:clen], a1T[:, hc, dc], xT[:, dc, csl],
                        start=(dc == 0), stop=(dc == DC - 1),
                    )
                p3 = hp.tile([128, 512], F32, tag="hps" if hp is h_ps else "y")
                for dc in range(DC):
                    nc.tensor.matmul(
                        p3[:, 0:clen], a3T[:, hc, dc], xT[:, dc, csl],
                        start=(dc == 0), stop=(dc == DC - 1),
                    )
                sl = silu_p.tile([128, 512], BF16, tag="silu")
                nc.scalar.activation(sl[:, 0:clen], p1[:, 0:clen], AF.Silu)
                nc.vector.tensor_tensor(
                    ht[:, hc, 0:clen], sl[:, 0:clen], p3[:, 0:clen], op=OP.mult
                )

            # y[d, t] = w2.T h for this piece (d-major output). The very
            # last d-chunk is split into halves so the final copy+DMA
            # overlap the final matmuls (shorter tail).
            last_piece = pi == len(pieces) - 1
            for dc in range(DC):
                nsp = (int(os.environ.get("K_TSPLIT", "2"))
                       if (last_piece and dc == DC - 1) else 1)
                step = -(-clen // nsp)
                for h0 in range(0, clen, step):
                    h1 = min(h0 + step, clen)
                    if (wup_ps is not y_ps
                            and os.environ.get("K_WUPY", "1") == "1"
                            and dc % 4 == 3):
                        py = wup_ps.tile([128, 512], F32, tag="wup")
                    else:
                        py = y_ps.tile([128, 512], F32, tag="y")
                    for hc in range(HC):
                        nc.tensor.matmul(
                            py[:, 0:h1 - h0], a2T[:, hc, dc], ht[:, hc, h0:h1],
                            start=(hc == 0), stop=(hc == HC - 1),
                        )
                    yo = yo_p.tile([128, 512], F32, tag="yo")
                    nc.vector.tensor_copy(yo[:, 0:h1 - h0], py[:, 0:h1 - h0])
                    nc.sync.dma_start(
                        y_d[:, dc, c0 + h0:c0 + h1], yo[:, 0:h1 - h0]
                    )


_CACHE = {}


def _wT_layout(w):
    """[HID, DIM] (bf16) -> DRAM layout [128, HC, DC, 128] where
    [p, hc, dc, i] = w[hc*128 + i, dc*128 + p]."""
    return np.ascontiguousarray(
        w.reshape(HC, 128, DC, 128).transpose(3, 0, 2, 1)
    )


def _w2T_layout(w):
    """[DIM, HID] (bf16) -> DRAM layout [128, HC, DC, 128] where
    [p, hc, dc, i] = w[dc*128 + i, hc*128 + p]."""
    return np.ascontiguousarray(
        w.T.reshape(HC, 128, DC, 128).transpose(1, 0, 2, 3)
    )


def _xT_layout(tok, N):
    """[N, DIM] (bf16) -> DRAM layout [128, DC, N]."""
    return np.ascontiguousarray(tok.T.reshape(DC, 128, N).transpose(1, 0, 2))


def kernel(x, gate_w, w1, w2, w3, ws1, ws2, ws3):
    x = np.asarray(x, dtype=np.float32)
    gate_w = np.asarray(gate_w, dtype=np.float32)
    w1 = np.asarray(w1, dtype=np.float32)
    w2 = np.asarray(w2, dtype=np.float32)
    w3 = np.asarray(w3, dtype=np.float32)
    ws1 = np.asarray(ws1, dtype=np.float32)
    ws2 = np.asarray(ws2, dtype=np.float32)
    ws3 = np.asarray(ws3, dtype=np.float32)

    B, S, D = x.shape
    x2 = np.ascontiguousarray(x.reshape(-1, D))
    Tn = x2.shape[0]
    assert Tn == T and D == DIM

    # --- gate: softmax + top-2 + weight normalization (host)
    logits = x2 @ gate_w.T
    m = logits.max(-1, keepdims=True)
    sm = np.exp(logits - m)
    sm /= sm.sum(-1, keepdims=True)
    ti = np.argsort(-sm, axis=-1)[:, :2]
    tw = np.take_along_axis(sm, ti, axis=-1)
    tw = tw / (tw.sum(-1, keepdims=True) + 1e-20)

    idx_e, cw_e = [], []
    for e in range(E):
        sel = (ti[:, 0] == e) | (ti[:, 1] == e)
        idx = np.nonzero(sel)[0]
        w_tok = np.where(ti[idx, 0] == e, tw[idx, 0], 0.0) + np.where(
            ti[idx, 1] == e, tw[idx, 1], 0.0
        )
        idx_e.append(idx)
        cw_e.append(w_tok.astype(np.float32))

    maxL = max(len(i) for i in idx_e)
    # A true flex segment (filling pad columns with shared work) costs a full
    # extra weight-set DMA per core and models worse (113.7us vs 105.8us);
    # kept only behind K_FLEX=1 for experiments.
    use_flex = os.environ.get("K_FLEX", "0") == "1" and maxL > 512
    if use_flex:
        A = 512
        F = -(-(maxL - A) // 8) * 8
        n_over = sum(1 for i in idx_e if len(i) > A)
        BSEG = -(-(T - (N_CORES - n_over) * F) // N_CORES // 8) * 8
    else:
        A = max(128, -(-maxL // 8) * 8)
        F = 0
        BSEG = B_SH
    N = A + F + BSEG

    key = ("nc", A, F, BSEG)
    if key not in _CACHE:
        _CACHE[key] = _build_kernel(A, F=F, B=BSEG)
    nc = _CACHE[key]
    _CACHE["nc"] = nc

    x_bf = x2.astype(bfloat16)
    sh_w = (
        _wT_layout(ws1.astype(bfloat16)),
        _wT_layout(ws3.astype(bfloat16)),
        _w2T_layout(ws2.astype(bfloat16)),
    )
    # distribute the shared-expert tokens: flex-shared cores get F tokens in
    # their flex segment, every core gets up to B in its shared segment
    sh_pos = 0
    in_maps = []
    core_meta = []
    for c in range(N_CORES):
        idx = idx_e[c]
        exp_n = min(len(idx), A)
        over_n = len(idx) - exp_n  # >0 only when use_flex and this expert overflows
        ew = (
            _wT_layout(w1[c].astype(bfloat16)),
            _wT_layout(w3[c].astype(bfloat16)),
            _w2T_layout(w2[c].astype(bfloat16)),
        )
        tok = np.zeros((N, DIM), dtype=bfloat16)
        tok[:exp_n] = x_bf[idx[:exp_n]]
        m = {
            "xT": None,
            "w1T": ew[0], "w3T": ew[1], "w2T": ew[2],
            "s1T": sh_w[0], "s3T": sh_w[1], "s2T": sh_w[2],
        }
        flex_sh_idx = None
        if F:
            if over_n:
                tok[A:A + over_n] = x_bf[idx[exp_n:]]
                m["f1T"], m["f3T"], m["f2T"] = ew
            else:
                fn = min(F, T - sh_pos)
                flex_sh_idx = np.arange(sh_pos, sh_pos + fn)
                tok[A:A + fn] = x_bf[flex_sh_idx]
                sh_pos += fn
                m["f1T"], m["f3T"], m["f2T"] = sh_w
        bn = min(BSEG, T - sh_pos)
        sh_idx = np.arange(sh_pos, sh_pos + bn)
        tok[A + F:A + F + bn] = x_bf[sh_idx]
        sh_pos += bn
        m["xT"] = _xT_layout(tok, N)
        in_maps.append(m)
        core_meta.append((exp_n, over_n, flex_sh_idx, sh_idx))
    assert sh_pos == T, f"shared token distribution bug: {sh_pos} != {T}"

    _CACHE["last_in_maps"] = in_maps
    res = run_bass_kernel_spmd(nc, in_maps, list(range(N_CORES)))

    y = np.zeros((T, DIM), dtype=np.float32)
    for c in range(N_CORES):
        yc_dm = np.asarray(res.results[c]["y"], dtype=np.float32)  # [128, DC, N]
        yc = yc_dm.transpose(1, 0, 2).reshape(DIM, N).T  # [N, DIM]
        idx = idx_e[c]
        exp_n, over_n, flex_sh_idx, sh_idx = core_meta[c]
        y[idx[:exp_n]] += cw_e[c][:exp_n, None] * yc[:exp_n]
        if over_n:
            y[idx[exp_n:]] += cw_e[c][exp_n:, None] * yc[A:A + over_n]
        elif flex_sh_idx is not None and len(flex_sh_idx):
            y[flex_sh_idx] += yc[A:A + len(flex_sh_idx)]
        y[sh_idx] += yc[A + F:A + F + len(sh_idx)]
    return y.reshape(B, S, D)


# revision 57
# speedup vs baseline: 1.0150x; 1.0002x over previous
"""Bass/Trainium2 kernel for nn_MOEFeedForward (8-expert top-2 MoE + shared expert).

Sharding: expert-parallel with host-side dispatch. The host computes the gate
(softmax + top-2) and routes tokens: core c receives expert c's tokens (padded
to capacity A = ceil8(max expert load)) plus a 1/8 token-slice of the
shared-expert work (B = 256 tokens). Every core runs A+B token-FFN columns of
identical shape (hid=2048, dim=768) — balanced, no 8x dense overcompute. The
host applies the gate weights and scatter-adds per-core outputs into the full
result.

Device kernel: all operands pre-transposed/laid out on the host so the device
does only contiguous DMAs and back-to-back bf16 matmuls at 1 col/cycle.
Column pieces of <=512 run mm1/mm3 (hid-chunked, PSUM-accumulated over the 6
dim-chunks), silu*mul drains to bf16 hT, then mm2 in d-major form
(y[d, t], 6 dim psums contracting 16 hid chunks). Dummy PE warmup matmuls
ramp the tensor-engine clock while the first DMAs land. Cost-model makespan
~105.8us/core vs ~99.8us pure-matmul floor at 2.4 GHz.

Self-contained: hardcodes shapes from the problem spec.
"""
import os
import sys

sys.path.insert(0, "/opt/trn_rl_repo")

from contextlib import ExitStack

import numpy as np
from ml_dtypes import bfloat16

import concourse.bass as bass
import concourse.tile as tile
from concourse import mybir
from concourse.bass_utils import run_bass_kernel_spmd
from concourse.vector_clock import ScopedClock

DIM = 768
HID = 2048
E = 8
T = 2048
N_CORES = 8
B_SH = T // N_CORES  # shared-expert tokens per core (256)
DC = DIM // 128      # 6 d-chunks
HC = HID // 128      # 16 hid-chunks

F32 = mybir.dt.float32
BF16 = mybir.dt.bfloat16

AF = mybir.ActivationFunctionType
OP = mybir.AluOpType


# ---------------------------------------------------------------------------
# Walrus in this container rejects CTRL instructions (NoOp/Drain) carrying
# more than one sem wait. TileContext's tail drain carries one wait per
# outstanding semaphore. Replace it with a chain of SP nops (one wait each)
# followed by a bare drain.
def _patched_drain_and_barrier(self, tick_clock, wait_clock):
    import bass_rust

    nop_inst = self.nc.sync.nop(nofuse=True, hint="pre_drain_wait_funnel")
    wait_clock.add_sem_waits(
        nop_inst.ins, ScopedClock({None: tick_clock.global_clock})
    )
    si = nop_inst.ins.sync_info
    waits = list(si.on_wait) if si else []
    if len(waits) > 1:
        nop_inst.ins.sync_info.on_wait = waits[:1]
        for w in waits[1:]:
            extra = self.nc.sync.nop(nofuse=True, hint="pre_drain_wait_funnel")
            extra.ins.sync_info = bass_rust.SyncInfo(on_wait=[w], on_update=[])
    self.nc.sync.drain()

    self.nc.all_engine_barrier()
    assert self.sems is not None
    popped = self.nc._tile_sem_poison_stack.pop()
    assert popped is self._sem_poison
    self.nc.clear_and_free_semaphores(list(self.sems.allocated().values()))
    self.nc.all_engine_barrier()


tile.TileContext._drain_and_barrier = _patched_drain_and_barrier


def _split_multi_waits(nc, max_waits=1):
    """This walrus build allows at most one sem wait per instruction. Hoist
    extra waits onto same-engine nops inserted immediately before."""
    import bass_rust

    n_split = 0
    for f in nc.m.functions:
        for bb in f.blocks:
            il = bb.instructions
            i = 0
            while i < len(il):
                inst = il[i]
                si = inst.sync_info
                if si is None or len(si.on_wait) <= max_waits:
                    i += 1
                    continue
                waits = list(si.on_wait)
                si.on_wait = waits[:max_waits]
                for k, w in enumerate(waits[max_waits:]):
                    nop = mybir.InstNoOp(
                        name=f"{inst.name}-wsplit{k}", ins=[], outs=[]
                    )
                    nop.engine = inst.engine
                    nop.sync_info = bass_rust.SyncInfo(on_wait=[w], on_update=[])
                    il.insert(i, nop)
                    i += 1
                n_split += 1
                i += 1
    return n_split
# ---------------------------------------------------------------------------


def _build_kernel(A, reps=1, F=0, B=B_SH):
    """A: expert-token capacity. Columns [0, A) use the expert weight set;
    with F>0, columns [A, A+F) use a per-core 'flex' weight set (host fills
    with either this core's expert weights or the shared weights); columns
    [A+F, A+F+B) use the shared weight set.
    Output y is d-major: y_d[p, dc, t] = y[t, dc*128+p].
    reps>1 repeats the whole compute (for benchmarking)."""
    N = A + F + B
    nc = bass.Bass()
    xT_d = nc.dram_tensor("xT", [128, DC, N], BF16, kind="ExternalInput")
    w1_d = nc.dram_tensor("w1T", [128, HC, DC, 128], BF16, kind="ExternalInput")
    w3_d = nc.dram_tensor("w3T", [128, HC, DC, 128], BF16, kind="ExternalInput")
    w2_d = nc.dram_tensor("w2T", [128, HC, DC, 128], BF16, kind="ExternalInput")
    s1_d = nc.dram_tensor("s1T", [128, HC, DC, 128], BF16, kind="ExternalInput")
    s3_d = nc.dram_tensor("s3T", [128, HC, DC, 128], BF16, kind="ExternalInput")
    s2_d = nc.dram_tensor("s2T", [128, HC, DC, 128], BF16, kind="ExternalInput")
    if F:
        f1_d = nc.dram_tensor("f1T", [128, HC, DC, 128], BF16, kind="ExternalInput")
        f3_d = nc.dram_tensor("f3T", [128, HC, DC, 128], BF16, kind="ExternalInput")
        f2_d = nc.dram_tensor("f2T", [128, HC, DC, 128], BF16, kind="ExternalInput")
    y_d = nc.dram_tensor("y", [128, DC, N], F32, kind="ExternalOutput")

    # column pieces: (start, len, weight-set)
    first = int(os.environ.get("K_P0", "512"))
    exp_pieces = []
    c0 = 0
    while c0 < A:
        ln = min(first if c0 == 0 else 512, A - c0)
        exp_pieces.append((c0, ln, 0))
        c0 += ln
    # flex is processed LAST: its weights reuse w1/w3/w2's SBUF (same pool
    # tags), so their DMA can only land after the expert pieces finish.
    pieces = exp_pieces + [(A + F, B, 1)]
    if F:
        pieces = pieces + [(A, F, 2)]

    _g = lambda k, d: int(os.environ.get(k, str(d)))
    with tile.TileContext(nc) as tc, ExitStack() as ctx:
        persist = ctx.enter_context(tc.tile_pool(name="persist", bufs=1))
        if os.environ.get("K_MERGE", "0") == "1":
            yo_p = ctx.enter_context(tc.tile_pool(name="yo", bufs=_g("K_YO", 3)))
            silu_p = yo_p
        else:
            silu_p = ctx.enter_context(
                tc.tile_pool(name="silu", bufs=_g("K_SILU", 3)))
            yo_p = ctx.enter_context(tc.tile_pool(name="yo", bufs=_g("K_YO", 3)))
        h5 = os.environ.get("K_H5", "1") == "1"
        h_ps = ctx.enter_context(
            tc.tile_pool(name="h_ps", bufs=_g("K_HPS", 5 if h5 else 4), space="PSUM"))
        y_ps = ctx.enter_context(
            tc.tile_pool(name="y_ps", bufs=_g("K_YPS", 3), space="PSUM"))
        if h5:
            wup_ps = y_ps  # warmup psum borrows the y ring; its bank goes to h_ps
        else:
            wup_ps = ctx.enter_context(
                tc.tile_pool(name="wup_ps", bufs=1, space="PSUM"))

        xT = persist.tile([128, DC, N], BF16, tag="xT")
        w1T = persist.tile([128, HC, DC, 128], BF16, tag="w1T")
        w3T = persist.tile([128, HC, DC, 128], BF16, tag="w3T")
        s1T = persist.tile([128, HC, DC, 128], BF16, tag="s1T")
        s3T = persist.tile([128, HC, DC, 128], BF16, tag="s3T")
        w2T = persist.tile([128, HC, DC, 128], BF16, tag="w2T")
        s2T = persist.tile([128, HC, DC, 128], BF16, tag="s2T")
        # hT holds one piece's activations [hid, piece_cols]; small pieces
        # (<=64 cols) get their own tile so piece transitions don't WAR-chain
        hT = persist.tile([128, HC, 512], BF16, tag="hT")
        hT2 = None
        if os.environ.get("K_HT2", "0") == "1":
            small = [p for p in pieces if p[1] <= 64]
            if small:
                hT2 = persist.tile([128, HC, max(p[1] for p in small)],
                                   BF16, tag="hT2")

        # --- PE warmup: dummy matmuls with no data deps keep the tensor
        # engine busy (and its clock ramping) while the first DMAs land.
        n_wup = _g("K_WUP", 6)
        if n_wup:
            wup = persist.tile([128, 512], BF16, tag="wup")
            nc.vector.memset(wup[:], 0)
            wup_p = wup_ps.tile([128, 512], F32, tag="wup" if not h5 else "y")
            for _ in range(n_wup):
                nc.tensor.matmul(wup_p[:], wup[:, 0:128], wup[:], start=True, stop=True)

        f_d = (f1_d, f3_d, f2_d) if F else None
        for _rep in range(reps):
            _kernel_body(nc, pieces, N, xT_d, w1_d, w3_d, w2_d, s1_d, s3_d,
                         s2_d, y_d, xT, w1T, w3T, s1T, s3T, w2T, s2T, hT,
                         silu_p, yo_p, h_ps, y_ps, persist, f_d, wup_ps, hT2)

    _split_multi_waits(nc)
    try:
        _CACHE["makespan_ns"] = max(e[2] for e in tc._perfetto_entries)
    except Exception:
        _CACHE["makespan_ns"] = None
    return nc


def _kernel_body(nc, pieces, N, xT_d, w1_d, w3_d, w2_d, s1_d, s3_d, s2_d,
                 y_d, xT, w1T, w3T, s1T, s3T, w2T, s2T, hT,
                 silu_p, yo_p, h_ps, y_ps, persist=None, f_d=None, wup_ps=None,
                 hT2=None):
    if True:
        # --- DMA schedule: first-piece x interleaved with first w1/w3
        # hid-chunks (PE consumes chunk-by-chunk), then the rest.
        c00, c0len = pieces[0][0], pieces[0][1]
        pro = os.environ.get("K_PRO", "0")
        if pro == "3":
            # one weight pair early (hc0 between x1 and x2), second pair after
            # all x — the first two h series then never stall on arrivals
            for dc in range(DC):
                nc.sync.dma_start(
                    xT[:, dc, c00:c00 + c0len], xT_d[:, dc, c00:c00 + c0len]
                )
                if dc == 1:
                    nc.sync.dma_start(w1T[:, 0], w1_d[:, 0])
                    nc.sync.dma_start(w3T[:, 0], w3_d[:, 0])
            nc.sync.dma_start(w1T[:, 1], w1_d[:, 1])
            nc.sync.dma_start(w3T[:, 1], w3_d[:, 1])
        elif pro == "2":
            # all of x first (PE is in warmup anyway), then w1/w3 pairs —
            # the first h series then never waits on an x chunk
            for dc in range(DC):
                nc.sync.dma_start(
                    xT[:, dc, c00:c00 + c0len], xT_d[:, dc, c00:c00 + c0len]
                )
            for hc in range(2):
                nc.sync.dma_start(w1T[:, hc], w1_d[:, hc])
                nc.sync.dma_start(w3T[:, hc], w3_d[:, hc])
        elif pro == "1":
            # w1 hc0 first so the first p1 series starts as x chunks stream in
            nc.sync.dma_start(w1T[:, 0], w1_d[:, 0])
            for dc in range(DC):
                nc.sync.dma_start(
                    xT[:, dc, c00:c00 + c0len], xT_d[:, dc, c00:c00 + c0len]
                )
            nc.sync.dma_start(w3T[:, 0], w3_d[:, 0])
            nc.sync.dma_start(w1T[:, 1], w1_d[:, 1])
            nc.sync.dma_start(w3T[:, 1], w3_d[:, 1])
        else:
            for dc in range(DC):
                nc.sync.dma_start(
                    xT[:, dc, c00:c00 + c0len], xT_d[:, dc, c00:c00 + c0len]
                )
                if dc < 2:
                    nc.sync.dma_start(w1T[:, dc], w1_d[:, dc])
                    nc.sync.dma_start(w3T[:, dc], w3_d[:, dc])
        for hc in range(2, HC):
            nc.sync.dma_start(w1T[:, hc], w1_d[:, hc])
            nc.sync.dma_start(w3T[:, hc], w3_d[:, hc])
            if hc == 7 and c0len < N:
                nc.sync.dma_start(xT[:, :, c0len:N], xT_d[:, :, c0len:N])
        nc.sync.dma_start(w2T[:], w2_d[:])
        nc.sync.dma_start(s1T[:], s1_d[:])
        nc.sync.dma_start(s3T[:], s3_d[:])
        nc.sync.dma_start(s2T[:], s2_d[:])
        if f_d is not None:
            # flex weights reuse the expert-weight SBUF slots (same tags,
            # bufs=1 ring): the DMAs wait for the expert pieces' last reads.
            f1T = persist.tile([128, HC, DC, 128], BF16, tag="w1T")
            f3T = persist.tile([128, HC, DC, 128], BF16, tag="w3T")
            f2T = persist.tile([128, HC, DC, 128], BF16, tag="w2T")
            for hc in range(HC):
                nc.sync.dma_start(f1T[:, hc], f_d[0][:, hc])
                nc.sync.dma_start(f3T[:, hc], f_d[1][:, hc])
            nc.sync.dma_start(f2T[:], f_d[2][:])

        n_borrow = int(os.environ.get("K_BORROW", "0"))
        psum_dma = os.environ.get("K_PSDMA", "0") == "1"
        for pi, (c0, clen, ws) in enumerate(pieces):
            if ws == 0:
                a1T, a3T, a2T = (w1T, w3T, w2T)
            elif ws == 1:
                a1T, a3T, a2T = (s1T, s3T, s2T)
            else:
                a1T, a3T, a2T = (f1T, f3T, f2T)
            ht = hT2 if (hT2 is not None and clen <= 64) else hT
            csl = slice(c0, c0 + clen)
            # h = silu(w1 x) * (w3 x), written to hT[:, :, 0:clen]
            for hc in range(HC):
                # y_ps banks are idle until the first mm2 — borrow them for
                # the first few h chunks so the startup WAR chain never waits
                hp = y_ps if (pi == 0 and hc < n_borrow) else h_ps
                p1 = hp.tile([128, 512], F32, tag="hps" if hp is h_ps else "y")
                for dc in range(DC):
                    nc.tensor.matmul(
                        p1[:, 0:clen], a1T[:, hc, dc], xT[:, dc, csl],
                        start=(dc == 0), stop=(dc == DC - 1),
                    )
                p3 = hp.tile([128, 512], F32, tag="hps" if hp is h_ps else "y")
                for dc in range(DC):
                    nc.tensor.matmul(
                        p3[:, 0:clen], a3T[:, hc, dc], xT[:, dc, csl],
                        start=(dc == 0), stop=(dc == DC - 1),
                    )
                sl = silu_p.tile([128, 512], BF16, tag="silu")
                nc.scalar.activation(sl[:, 0:clen], p1[:, 0:clen], AF.Silu)
                nc.vector.tensor_tensor(
                    ht[:, hc, 0:clen], sl[:, 0:clen], p3[:, 0:clen], op=OP.mult
                )

            # y[d, t] = w2.T h for this piece (d-major output). The very
            # last d-chunk is split into halves so the final copy+DMA
            # overlap the final matmuls (shorter tail).
            last_piece = pi == len(pieces) - 1
            for dc in range(DC):
                if last_piece and dc == DC - 1:
                    tl = int(os.environ.get("K_TLAST", "96"))
                    if tl and clen > tl:
                        bounds = [(0, clen - tl), (clen - tl, clen)]
                    else:
                        nsp = int(os.environ.get("K_TSPLIT", "2"))
                        step = -(-clen // nsp)
                        bounds = [(h0, min(h0 + step, clen))
                                  for h0 in range(0, clen, step)]
                else:
                    bounds = [(0, clen)]
                for h0, h1 in bounds:
                    if (wup_ps is not y_ps
                            and os.environ.get("K_WUPY", "1") == "1"
                            and dc % 4 == 3):
                        py = wup_ps.tile([128, 512], F32, tag="wup")
                    else:
                        py = y_ps.tile([128, 512], F32, tag="y")
                    for hc in range(HC):
                        nc.tensor.matmul(
                            py[:, 0:h1 - h0], a2T[:, hc, dc], ht[:, hc, h0:h1],
                            start=(hc == 0), stop=(hc == HC - 1),
                        )
                    yo = yo_p.tile([128, 512], F32, tag="yo")
                    nc.vector.tensor_copy(yo[:, 0:h1 - h0], py[:, 0:h1 - h0])
                    nc.sync.dma_start(
                        y_d[:, dc, c0 + h0:c0 + h1], yo[:, 0:h1 - h0]
                    )


_CACHE = {}


def _wT_layout(w):
    """[HID, DIM] (bf16) -> DRAM layout [128, HC, DC, 128] where
    [p, hc, dc, i] = w[hc*128 + i, dc*128 + p]."""
    return np.ascontiguousarray(
        w.reshape(HC, 128, DC, 128).transpose(3, 0, 2, 1)
    )


def _w2T_layout(w):
    """[DIM, HID] (bf16) -> DRAM layout [128, HC, DC, 128] where
    [p, hc, dc, i] = w[dc*128 + i, hc*128 + p]."""
    return np.ascontiguousarray(
        w.T.reshape(HC, 128, DC, 128).transpose(1, 0, 2, 3)
    )


def _xT_layout(tok, N):
    """[N, DIM] (bf16) -> DRAM layout [128, DC, N]."""
    return np.ascontiguousarray(tok.T.reshape(DC, 128, N).transpose(1, 0, 2))


def kernel(x, gate_w, w1, w2, w3, ws1, ws2, ws3):
    x = np.asarray(x, dtype=np.float32)
    gate_w = np.asarray(gate_w, dtype=np.float32)
    w1 = np.asarray(w1, dtype=np.float32)
    w2 = np.asarray(w2, dtype=np.float32)
    w3 = np.asarray(w3, dtype=np.float32)
    ws1 = np.asarray(ws1, dtype=np.float32)
    ws2 = np.asarray(ws2, dtype=np.float32)
    ws3 = np.asarray(ws3, dtype=np.float32)

    B, S, D = x.shape
    x2 = np.ascontiguousarray(x.reshape(-1, D))
    Tn = x2.shape[0]
    assert Tn == T and D == DIM

    # --- gate: softmax + top-2 + weight normalization (host)
    logits = x2 @ gate_w.T
    m = logits.max(-1, keepdims=True)
    sm = np.exp(logits - m)
    sm /= sm.sum(-1, keepdims=True)
    ti = np.argsort(-sm, axis=-1)[:, :2]
    tw = np.take_along_axis(sm, ti, axis=-1)
    tw = tw / (tw.sum(-1, keepdims=True) + 1e-20)

    idx_e, cw_e = [], []
    for e in range(E):
        sel = (ti[:, 0] == e) | (ti[:, 1] == e)
        idx = np.nonzero(sel)[0]
        w_tok = np.where(ti[idx, 0] == e, tw[idx, 0], 0.0) + np.where(
            ti[idx, 1] == e, tw[idx, 1], 0.0
        )
        idx_e.append(idx)
        cw_e.append(w_tok.astype(np.float32))

    maxL = max(len(i) for i in idx_e)
    # A true flex segment (filling pad columns with shared work) costs a full
    # extra weight-set DMA per core and models worse (113.7us vs 105.8us);
    # kept only behind K_FLEX=1 for experiments.
    use_flex = os.environ.get("K_FLEX", "0") == "1" and maxL > 512
    if use_flex:
        A = 512
        F = -(-(maxL - A) // 8) * 8
        n_over = sum(1 for i in idx_e if len(i) > A)
        BSEG = -(-(T - (N_CORES - n_over) * F) // N_CORES // 8) * 8
    else:
        A = max(128, -(-maxL // 8) * 8)
        F = 0
        BSEG = B_SH
    N = A + F + BSEG

    key = ("nc", A, F, BSEG)
    if key not in _CACHE:
        _CACHE[key] = _build_kernel(A, F=F, B=BSEG)
    nc = _CACHE[key]
    _CACHE["nc"] = nc

    x_bf = x2.astype(bfloat16)
    sh_w = (
        _wT_layout(ws1.astype(bfloat16)),
        _wT_layout(ws3.astype(bfloat16)),
        _w2T_layout(ws2.astype(bfloat16)),
    )
    # distribute the shared-expert tokens: flex-shared cores get F tokens in
    # their flex segment, every core gets up to B in its shared segment
    sh_pos = 0
    in_maps = []
    core_meta = []
    for c in range(N_CORES):
        idx = idx_e[c]
        exp_n = min(len(idx), A)
        over_n = len(idx) - exp_n  # >0 only when use_flex and this expert overflows
        ew = (
            _wT_layout(w1[c].astype(bfloat16)),
            _wT_layout(w3[c].astype(bfloat16)),
            _w2T_layout(w2[c].astype(bfloat16)),
        )
        tok = np.zeros((N, DIM), dtype=bfloat16)
        tok[:exp_n] = x_bf[idx[:exp_n]]
        m = {
            "xT": None,
            "w1T": ew[0], "w3T": ew[1], "w2T": ew[2],
            "s1T": sh_w[0], "s3T": sh_w[1], "s2T": sh_w[2],
        }
        flex_sh_idx = None
        if F:
            if over_n:
                tok[A:A + over_n] = x_bf[idx[exp_n:]]
                m["f1T"], m["f3T"], m["f2T"] = ew
            else:
                fn = min(F, T - sh_pos)
                flex_sh_idx = np.arange(sh_pos, sh_pos + fn)
                tok[A:A + fn] = x_bf[flex_sh_idx]
                sh_pos += fn
                m["f1T"], m["f3T"], m["f2T"] = sh_w
        bn = min(BSEG, T - sh_pos)
        sh_idx = np.arange(sh_pos, sh_pos + bn)
        tok[A + F:A + F + bn] = x_bf[sh_idx]
        sh_pos += bn
        m["xT"] = _xT_layout(tok, N)
        in_maps.append(m)
        core_meta.append((exp_n, over_n, flex_sh_idx, sh_idx))
    assert sh_pos == T, f"shared token distribution bug: {sh_pos} != {T}"

    _CACHE["last_in_maps"] = in_maps
    res = run_bass_kernel_spmd(nc, in_maps, list(range(N_CORES)))

    y = np.zeros((T, DIM), dtype=np.float32)
    for c in range(N_CORES):
        yc_dm = np.asarray(res.results[c]["y"], dtype=np.float32)  # [128, DC, N]
        yc = yc_dm.transpose(1, 0, 2).reshape(DIM, N).T  # [N, DIM]
        idx = idx_e[c]
        exp_n, over_n, flex_sh_idx, sh_idx = core_meta[c]
        y[idx[:exp_n]] += cw_e[c][:exp_n, None] * yc[:exp_n]
        if over_n:
            y[idx[exp_n:]] += cw_e[c][exp_n:, None] * yc[A:A + over_n]
        elif flex_sh_idx is not None and len(flex_sh_idx):
            y[flex_sh_idx] += yc[A:A + len(flex_sh_idx)]
        y[sh_idx] += yc[A + F:A + F + len(sh_idx)]
    return y.reshape(B, S, D)
